# revision 1
# baseline (speedup 1.0000x reference)
"""Trainium2 Bass kernel for GroupedQueryAttention with 1-bit quantized linears.

Sharding: 8 cores = 2 batches x 4 token-interleaved groups.
Core c handles batch b=c//4 and tokens t with t%4 == i (i=c%4), i.e. 512
query tokens per core.  Every core computes full K/V for its batch
(replicated), all 16 heads for its own 512 queries, and the full O
projection for those rows.  Host gathers by re-interleaving rows.

Optimizations vs the original baseline (822us -> ~565us):
 - x/xq ship from host as bf16; the device only DMA-transposes them
   (XBAR) into din-major SBUF layout, no cast pass or DRAM round-trip.
 - interleaved emission: Q head h quantizes+projects while x chunk c
   transposes; K chunk c projects after head 4c+3, V+o-quant after, so
   the PE stream never waits long on staging.
 - weight quant chain spread across engines: sync HW-DGE f32 load ->
   DVE abs-group-reduce -> ACT sign -> gpsimd scale-mult -> sync
   DMA-transpose.
 - rope: ACT evacuates PSUM (fast PSUM port), DVE multiplies against
   f16 tables (sin table pre-rolled 64 partitions for base-partition
   legality), gpsimd adds.
 - attention: scores for 2 key-tiles land in one 2-bank PSUM tile and
   share one strided exp op (halves ACT per-op overhead); the softmax
   denominator matmul uses a [128,128] ones stationary so the sums
   arrive broadcast on all partitions -> wide reciprocal_approx_fast
   (no single-partition crawl, no DRAM broadcast trip).
 - o-weight quant rides along V proj; owT transposes overlap attention.

Program is identical across cores; all per-core variation is input data.
"""

import sys

sys.path.insert(0, "/opt/trn_rl_repo")

import numpy as np
import ml_dtypes

import concourse.bacc as bacc
import concourse.bass as bass
import concourse.mybir as mybir
import concourse.tile as tile

F32 = mybir.dt.float32
F16 = mybir.dt.float16
BF16 = mybir.dt.bfloat16

B, T, D = 2, 2048, 2048
H, HK, HD = 16, 4, 128
G = 128
THETA = 1000000.0
NC = 8
TQ = T // 4          # 512 query tokens per core
QT = TQ // 128       # 4 query tiles
DT = D // 128        # 16 din tiles
NKT = T // 128       # 16 key tiles
NPAIR = NKT // 2     # 8 key-tile pairs

ALPHA_K = 1.0 / G
ALPHA_Q = (HD ** -0.5) / G


def _bcast(ap_small, like_ap):
    a, b = bass.broadcast_tensor_aps(like_ap, ap_small)
    return b


def build_program():
    nc = bacc.Bacc("TRN2", target_bir_lowering=False, debug=False, num_devices=NC)

    x = nc.dram_tensor("x", [D, T], BF16, kind="ExternalInput").ap()
    xq = nc.dram_tensor("xq", [D, TQ], BF16, kind="ExternalInput").ap()
    qw = nc.dram_tensor("qw", [H * HD, D], BF16, kind="ExternalInput").ap()
    kw = nc.dram_tensor("kw", [HK * HD, D], BF16, kind="ExternalInput").ap()
    vw = nc.dram_tensor("vw", [HK * HD, D], BF16, kind="ExternalInput").ap()
    ow = nc.dram_tensor("ow", [D, H * HD], BF16, kind="ExternalInput").ap()
    cosk = nc.dram_tensor("cosk", [HD, T], F16, kind="ExternalInput").ap()
    sinkr = nc.dram_tensor("sinkr", [HD, T], F16, kind="ExternalInput").ap()
    cosq = nc.dram_tensor("cosq", [HD, TQ], F16, kind="ExternalInput").ap()
    sinqr = nc.dram_tensor("sinqr", [HD, TQ], F16, kind="ExternalInput").ap()
    dmask = nc.dram_tensor("dmask", [128, 32], BF16, kind="ExternalInput").ap()
    out = nc.dram_tensor("out", [TQ, D], F32, kind="ExternalOutput").ap()

    with tile.TileContext(nc) as tc:
        build_tile_kernel(nc, tc, x, xq, qw, kw, vw, ow, cosk, sinkr, cosq,
                          sinqr, dmask, out)
    nc.compile()
    return nc


def build_tile_kernel(nc, tc, x, xq, qw, kw, vw, ow, cosk, sinkr, cosq, sinqr,
                      dmask, out):
    from contextlib import ExitStack

    ctx = ExitStack()
    with ctx:
        # ------- long-lived pools --------
        dram = ctx.enter_context(tc.tile_pool(name="dram", bufs=1, space="DRAM"))
        const = ctx.enter_context(tc.tile_pool(name="const", bufs=1))
        resid = ctx.enter_context(tc.tile_pool(name="resid", bufs=1))

        wqd = dram.tile([D, H * HD], BF16)      # quantized o-weights (row major)

        dmask_sb = const.tile([128, 32], BF16)
        nc.sync.dma_start(dmask_sb, dmask)
        # [128,128] of G*G: sum-matmul output lands broadcast on all 128
        # partitions, so the reciprocal runs on 128 lanes (no [1,512] crawl)
        ones16k = const.tile([128, 128], BF16)
        nc.gpsimd.memset(ones16k, float(G * G))

        cosk_sb = const.tile([128, T], F16)
        sinkr_sb = const.tile([128, T], F16)
        cosq_sb = const.tile([128, TQ], F16)
        sinqr_sb = const.tile([128, TQ], F16)
        nc.sync.dma_start(cosk_sb, cosk)
        nc.sync.dma_start(sinkr_sb, sinkr)
        nc.sync.dma_start(cosq_sb, cosq)
        nc.sync.dma_start(sinqr_sb, sinqr)

        # residents alive through attention (48 KB/partition)
        QT_t = resid.tile([128, H, TQ], BF16)    # roped q^T  [d, h, t]
        KT_t = resid.tile([128, HK, T], BF16)    # roped k^T  [d, hk, t]
        V_t = resid.tile([128, NKT, HK * HD], BF16)  # v [t, kv-dim]

        # ============== phase 1: staging + projections ==================
        with tc.tile_pool(name="xtp", bufs=1) as pxt, \
             tc.tile_pool(name="wst", bufs=3) as wst_p, \
             tc.tile_pool(name="wqt", bufs=3) as wqt_p, \
             tc.tile_pool(name="ssum", bufs=2) as ssum_p, \
             tc.tile_pool(name="rtmp", bufs=2) as rtmp, \
             tc.tile_pool(name="proj_ps", bufs=4, space="PSUM") as pps:

            XT = pxt.tile([128, DT, T], BF16)     # x^T (din-major), 64 KB/p

            def quant_tile(w_ap, row_tile, out_T=None, out_rowmajor=None):
                """Load 128 rows of f32 w, 1-bit quantize -> bf16 (x G scale).

                sign (ACT) writes into wqt, then DVE scales in place.
                """
                wst = wst_p.tile([128, D], BF16, tag="wst")
                nc.sync.dma_start(
                    wst, w_ap[row_tile * 128:(row_tile + 1) * 128, :])
                ssum = ssum_p.tile([128, DT], F32, tag="ssum")
                nc.vector.tensor_reduce(
                    ssum, wst.rearrange("p (g c) -> p g c", c=G),
                    axis=mybir.AxisListType.X, op=mybir.AluOpType.add,
                    apply_absolute_value=True)
                wqt = wqt_p.tile([128, D], BF16, tag="wqt")
                nc.scalar.sign(wqt, wst)
                sv = ssum.rearrange("p (g o) -> p g o", o=1)
                gv = wqt.rearrange("p (g c) -> p g c", c=G)
                nc.gpsimd.tensor_tensor(gv, gv, _bcast(sv, gv),
                                        op=mybir.AluOpType.mult)
                if out_T is not None:
                    nc.sync.dma_start_transpose(out_T, wqt[:])
                if out_rowmajor is not None:
                    nc.sync.dma_start(out_rowmajor, wqt[:])


            def rope_evac(ps, cos_sb, sinr_sb, col0, width, out_ap):
                """out = ps*cos + rot(ps)*sinr  (cast bf16).

                ACT evacuates PSUM first (it has the fast PSUM port); DVE
                multiplies run on SBUF at full rate.
                """
                pse = rtmp.tile([128, width], F32, tag="pse", bufs=2)
                nc.scalar.copy(pse, ps)
                t1 = rtmp.tile([128, width], F32, tag="t1", bufs=1)
                t2 = rtmp.tile([128, width], F32, tag="t2", bufs=1)
                cs = cos_sb[:, col0:col0 + width]
                sr = sinr_sb[:, col0:col0 + width]
                # sinr tables arrive pre-rolled by 64 partitions so that the
                # rotate-half reads share a base partition with the table
                nc.vector.tensor_tensor(t1, pse, cs, op=mybir.AluOpType.mult)
                nc.vector.tensor_tensor(t2[0:64, :], pse[64:128, :],
                                        sr[64:128, :], op=mybir.AluOpType.mult)
                nc.vector.tensor_tensor(t2[64:128, :], pse[0:64, :],
                                        sr[0:64, :], op=mybir.AluOpType.mult)
                nc.gpsimd.tensor_tensor(out_ap, t1, t2, op=mybir.AluOpType.add)

            # --- interleaved: xq staging, Q heads, x staging, K chunks ---
            # x casts to bf16 in DRAM (gpsimd cast-DMA, no engine work), then
            # DRAM->SBUF transposes per 512-token chunk; K chunk c projects
            # after head 4c+3 so PE never waits on staging.
            with tc.tile_pool(name="qk", bufs=1) as qk_p:
                XTq = qk_p.tile([128, DT, TQ], BF16, tag="XTq", bufs=1)
                # x/xq arrive bf16 AND pre-transposed (din-major) from the
                # host: plain strided loads, no XBAR transposes at all
                nc.sync.dma_start(
                    XTq[:], xq.rearrange("(dt p) t -> p dt t", p=128))

                kwT = []
                for h in range(H):
                    if h % 4 == 0:
                        c = h // 4
                        nc.sync.dma_start(
                            XT[:, :, c * 512:(c + 1) * 512],
                            x.rearrange("(dt p) t -> p dt t",
                                        p=128)[:, :, c * 512:(c + 1) * 512])
                    wT = qk_p.tile([128, DT, 128], BF16, tag="qwT", bufs=2)
                    quant_tile(qw, h, out_T=wT[:])
                    ps = pps.tile([128, TQ], F32, tag="ps")
                    for dt in range(DT):
                        nc.tensor.matmul(ps, wT[:, dt, :], XTq[:, dt, :],
                                         start=(dt == 0), stop=(dt == DT - 1))
                    rope_evac(ps, cosq_sb, sinqr_sb, 0, TQ, QT_t[:, h, :])

                    if h == 0:
                        for hk in range(HK):
                            wTk = qk_p.tile([128, DT, 128], BF16, tag="kwT",
                                            bufs=4)
                            quant_tile(kw, hk, out_T=wTk[:])
                            kwT.append(wTk)

                    if h % 4 == 3:
                        tc4 = h // 4
                        for hk in range(HK):
                            ps = pps.tile([128, 512], F32, tag="ps")
                            for dt in range(DT):
                                nc.tensor.matmul(
                                    ps, kwT[hk][:, dt, :],
                                    XT[:, dt, tc4 * 512:(tc4 + 1) * 512],
                                    start=(dt == 0), stop=(dt == DT - 1))
                            rope_evac(ps, cosk_sb, sinkr_sb, tc4 * 512, 512,
                                      KT_t[:, hk, tc4 * 512:(tc4 + 1) * 512])

            # --- V projection + o-weight quant ---
            with tc.tile_pool(name="vq", bufs=1) as vq_p:
                vq = vq_p.tile([128, DT, HK * HD], BF16)
                for rv in range(HK * HD // 128):
                    quant_tile(vw, rv, out_T=vq[:, :, rv * 128:(rv + 1) * 128])
                for tch in range(NKT):
                    ps = pps.tile([128, HK * HD], F32, tag="ps")
                    for dt in range(DT):
                        nc.tensor.matmul(ps,
                                         XT[:, dt, tch * 128:(tch + 1) * 128],
                                         vq[:, dt, :],
                                         start=(dt == 0), stop=(dt == DT - 1))
                    nc.scalar.copy(V_t[:, tch, :], ps)
                    # o-weight quant engine work rides along with V proj
                    quant_tile(ow, tch,
                               out_rowmajor=wqd[tch * 128:(tch + 1) * 128, :])

        # ============== phase 2: attention + output projection ==========
        with tc.tile_pool(name="att_res", bufs=1) as ares:

            OT_t = ares.tile([128, H, TQ], BF16)   # attn out^T [dv, h, q]
            owT = ares.tile([128, H, D], BF16)     # o-weights^T [dH, ht, dout]

            with tc.tile_pool(name="attn", bufs=2) as apool, \
                 tc.tile_pool(name="st_ps", bufs=2, space="PSUM") as stp, \
                 tc.tile_pool(name="sum_ps", bufs=1, space="PSUM") as sump, \
                 tc.tile_pool(name="o_ps", bufs=2, space="PSUM") as op:
                attention_heads(nc, tc, apool, stp, sump, op, KT_t, QT_t, V_t,
                                OT_t, owT, wqd, dmask_sb, ones16k)

            # --- output projection ---
            with tc.tile_pool(name="oproj", bufs=2) as opool, \
                 tc.tile_pool(name="op_ps", bufs=4, space="PSUM") as opp:
                for m in range(QT):
                    osb = opool.tile([128, D], F32, tag="osb")
                    for oc in range(4):
                        ps = opp.tile([128, 512], F32, tag="ps")
                        for ht in range(H):
                            nc.tensor.matmul(ps,
                                             OT_t[:, ht, m * 128:(m + 1) * 128],
                                             owT[:, ht, oc * 512:(oc + 1) * 512],
                                             start=(ht == 0), stop=(ht == H - 1))
                        if oc % 2 == 0:
                            nc.vector.tensor_copy(
                                osb[:, oc * 512:(oc + 1) * 512], ps)
                        else:
                            nc.scalar.copy(osb[:, oc * 512:(oc + 1) * 512], ps)
                    nc.sync.dma_start(out[m * 128:(m + 1) * 128, :], osb)


def attention_heads(nc, tc, apool, stp, sump, op, KT_t, QT_t, V_t, OT_t, owT,
                    wqd, dmask_sb, ones16k):
            for h in range(H):
                hk = h // 4
                ps_o = op.tile([128, TQ], F32, tag="ps_o")
                ps_sum = sump.tile([128, TQ], F32, tag="ps_sum")
                for p in range(NPAIR):
                    kt0, kt1 = 2 * p, 2 * p + 1
                    q0, q1 = 32 * kt0, 32 * kt1
                    ps_st = stp.tile([128, 2 * TQ], F32, tag="ps_st")
                    nc.tensor.matmul(ps_st[:, q0:TQ],
                                     KT_t[:, hk, kt0 * 128:(kt0 + 1) * 128],
                                     QT_t[:, h, q0:], start=True, stop=True)
                    nc.tensor.matmul(ps_st[:, TQ + q1:2 * TQ],
                                     KT_t[:, hk, kt1 * 128:(kt1 + 1) * 128],
                                     QT_t[:, h, q1:], start=True, stop=True)
                    pt = apool.tile([128, 2 * TQ], BF16, tag="pt", bufs=4)
                    # one exp op over both halves, strided to skip the gap
                    nc.scalar.activation(
                        pt.rearrange("p (k q) -> p k q", k=2)[:, :, q0:],
                        ps_st.rearrange("p (k q) -> p k q", k=2)[:, :, q0:],
                        mybir.ActivationFunctionType.Exp)
                    # diagonal strip masks (multiplicative 0/1)
                    nc.gpsimd.tensor_tensor(pt[:, q0:q0 + 32], pt[:, q0:q0 + 32],
                                            dmask_sb, op=mybir.AluOpType.mult)
                    nc.gpsimd.tensor_tensor(pt[:, TQ + q1:TQ + q1 + 32],
                                            pt[:, TQ + q1:TQ + q1 + 32],
                                            dmask_sb, op=mybir.AluOpType.mult)
                    # denominator + attn@V accumulation
                    nc.tensor.matmul(ps_sum[:, q0:], ones16k, pt[:, q0:TQ],
                                     start=(p == 0), stop=False)
                    nc.tensor.matmul(ps_sum[:, q1:], ones16k,
                                     pt[:, TQ + q1:2 * TQ],
                                     start=False, stop=(p == NPAIR - 1))
                    nc.tensor.matmul(ps_o[:, q0:],
                                     V_t[:, kt0, hk * HD:(hk + 1) * HD],
                                     pt[:, q0:TQ], start=(p == 0), stop=False)
                    nc.tensor.matmul(ps_o[:, q1:],
                                     V_t[:, kt1, hk * HD:(hk + 1) * HD],
                                     pt[:, TQ + q1:2 * TQ],
                                     start=False, stop=(p == NPAIR - 1))
                # sums arrive broadcast on all 128 partitions: fast reciprocal
                RQb = apool.tile([128, TQ], F32, tag="RQb", bufs=2)
                nc.vector.reciprocal_approx_fast(RQb, ps_sum)
                nc.vector.tensor_tensor(OT_t[:, h, :], ps_o, RQb,
                                        op=mybir.AluOpType.mult)
                # interleave owT transposes with attention
                if h >= 8:
                    for ht in (2 * (h - 8), 2 * (h - 8) + 1):
                        nc.sync.dma_start_transpose(
                            owT[:, ht, :], wqd[:, ht * 128:(ht + 1) * 128])


# ---------------------------------------------------------------------------
# host side
# ---------------------------------------------------------------------------
_CACHE = {}


def _tables():
    inv = 1.0 / (THETA ** (np.arange(0, HD, 2, dtype=np.float64) / HD))
    t = np.arange(T, dtype=np.float64)
    fr = np.outer(t, inv)                      # [T, 64]
    emb = np.concatenate([fr, fr], axis=1)     # [T, 128]
    cosT = np.cos(emb).T                       # [128, T] float64
    sinT = np.sin(emb).T
    sinr = np.empty_like(sinT)
    sinr[0:64] = -sinT[0:64]
    sinr[64:128] = sinT[64:128]
    # rolled by 64 partitions: kernel reads srs[64:128] for out[0:64] etc.
    sinr = np.roll(sinr, 64, axis=0)
    return cosT, sinT, sinr


def make_in_maps(hidden, q_w, k_w, v_w, o_w):
    cosT, sinT, sinr = _tables()
    f16 = np.float16
    in_maps = []
    for c in range(NC):
        b, i = c // 4, c % 4
        xb_ = np.ascontiguousarray(hidden[b].T.astype(ml_dtypes.bfloat16))
        xq_ = np.ascontiguousarray(hidden[b][i::4, :].T.astype(ml_dtypes.bfloat16))
        cq = np.ascontiguousarray(cosT[:, i::4] * ALPHA_Q).astype(f16)
        sq = np.ascontiguousarray(sinr[:, i::4] * ALPHA_Q).astype(f16)
        # dmask[r, c] = 1 iff key-local r <= 4c + i (diagonal 128x32 strip)
        r = np.arange(128)[:, None]
        cc = np.arange(32)[None, :]
        dm = (r <= 4 * cc + i).astype(ml_dtypes.bfloat16)
        bf = ml_dtypes.bfloat16
        in_maps.append({
            "x": xb_, "xq": xq_, "qw": q_w.astype(bf), "kw": k_w.astype(bf),
            "vw": v_w.astype(bf), "ow": o_w.astype(bf),
            "cosk": np.ascontiguousarray(cosT * ALPHA_K).astype(f16),
            "sinkr": np.ascontiguousarray(sinr * ALPHA_K).astype(f16),
            "cosq": cq, "sinqr": sq, "dmask": dm,
        })
    return in_maps


def kernel(hidden, q_w, k_w, v_w, o_w):
    hidden = np.asarray(hidden, dtype=np.float32)
    q_w = np.ascontiguousarray(np.asarray(q_w, dtype=np.float32))
    k_w = np.ascontiguousarray(np.asarray(k_w, dtype=np.float32))
    v_w = np.ascontiguousarray(np.asarray(v_w, dtype=np.float32))
    o_w = np.ascontiguousarray(np.asarray(o_w, dtype=np.float32))

    if "nc" not in _CACHE:
        _CACHE["nc"] = build_program()
    nc = _CACHE["nc"]

    in_maps = make_in_maps(hidden, q_w, k_w, v_w, o_w)
    from concourse.bass_utils import run_bass_kernel_spmd
    res = run_bass_kernel_spmd(nc, in_maps, core_ids=list(range(NC)))
    out = np.empty((B, T, D), dtype=np.float32)
    for c in range(NC):
        b, i = c // 4, c % 4
        out[b, i::4, :] = res.results[c]["out"]
    return out


if __name__ == "__main__":
    print("building program...")
    nc = build_program()
    print("BUILD OK")



# revision 10
# speedup vs baseline: 1.9152x; 1.9152x over previous
"""Trainium2 Bass kernel for GroupedQueryAttention with 1-bit quantized linears.

Sharding (v2): 8 cores = 2 batches x 4 head-groups (tensor-parallel over
heads).  Core c handles batch b=c//4 and head-group hg=c%4: query heads
4hg..4hg+3, kv head hg, ALL 2048 tokens.  The output projection is computed
over the local 512 attention-output columns only -> each core emits a partial
[T, D] sum; the host adds the 4 partials per batch.  No K/V compute
replication (the v1 baseline recomputed full K/V on 4 cores each).

Per-core FLOPs drop 23.6 -> 17.2 GF and every matmul runs 512 output
columns (single PSUM bank) with contraction 128, emitted as one dense
back-to-back PE stream:

 - The PE HAM clock gate defaults to 1.2 GHz and only reaches 2.4 GHz after
   ~3.4us of continuous busy; idle windows re-throttle.  The v1 kernel ran
   mostly cold.  Here the PE stream is kept dense (software-pipelined
   attention, per-512-chunk projection evacuation, double-buffered PSUM) and
   dummy matmuls fill the DMA-bound first ~30us to hold the gate open.
 - Weights arrive host-pre-transposed (din-major); 1-bit quantization runs
   in that layout: scale = ones-matmul partition-reduce of |w| (broadcast to
   all partitions), then sign (ACT) * scale (DVE) in place.  No DRAM round
   trip, no on-device weight transposes.
 - Attention per (head, 512-query quarter): scores[k,q] via K-tile
   stationary, one strided exp over both heads' PSUM banks, triangular
   dmask on diagonal tiles, softmax denominator via ones-matmul (sum lands
   broadcast on all partitions -> wide fast reciprocal), V accumulation in
   PSUM, normalize on evac.
 - O-projection: OT stationary x quantized owT moving, 4x512-col chunks,
   bf16 partial out streamed to DRAM per 128-row tile.

Program is identical across cores; all per-core variation is input data.
"""

import sys

sys.path.insert(0, "/opt/trn_rl_repo")

import numpy as np
import ml_dtypes

import concourse.bacc as bacc
import concourse.bass as bass
import concourse.mybir as mybir
import concourse.tile as tile

F32 = mybir.dt.float32
F16 = mybir.dt.float16
BF16 = mybir.dt.bfloat16

B, T, D = 2, 2048, 2048
H, HK, HD = 16, 4, 128
G = 128
THETA = 1000000.0
NC = 8
HL = H // 4          # 4 local query heads per core
DT = D // 128        # 16 din tiles
NKT = T // 128       # 16 key tiles
NQC = T // 512       # 4 query quarters

ALPHA_Q = HD ** -0.5

Exp = mybir.ActivationFunctionType.Exp
Abs = mybir.ActivationFunctionType.Abs
MULT = mybir.AluOpType.mult
ADD = mybir.AluOpType.add
ABSMAX = mybir.AluOpType.abs_max


def _bcast(ap_small, like_ap):
    a, b = bass.broadcast_tensor_aps(like_ap, ap_small)
    return b


def build_program():
    nc = bacc.Bacc("TRN2", target_bir_lowering=False, debug=False, num_devices=NC)

    xT = nc.dram_tensor("xT", [D, T], BF16, kind="ExternalInput").ap()
    qwT = nc.dram_tensor("qwT", [D, HL * HD], BF16, kind="ExternalInput").ap()
    kwT = nc.dram_tensor("kwT", [D, HD], BF16, kind="ExternalInput").ap()
    vwT = nc.dram_tensor("vwT", [D, HD], BF16, kind="ExternalInput").ap()
    owT = nc.dram_tensor("owT", [HL * HD, D], BF16, kind="ExternalInput").ap()
    cosq = nc.dram_tensor("cosq", [HD, T], F16, kind="ExternalInput").ap()
    sinqr = nc.dram_tensor("sinqr", [HD, T], F16, kind="ExternalInput").ap()
    cosk = nc.dram_tensor("cosk", [HD, T], F16, kind="ExternalInput").ap()
    sinkr = nc.dram_tensor("sinkr", [HD, T], F16, kind="ExternalInput").ap()
    trimask = nc.dram_tensor("trimask", [128, 128], BF16, kind="ExternalInput").ap()
    out = nc.dram_tensor("out", [T, D], BF16, kind="ExternalOutput").ap()

    with tile.TileContext(nc) as tc:
        build_tile_kernel(nc, tc, xT, qwT, kwT, vwT, owT, cosq, sinqr, cosk,
                          sinkr, trimask, out)
    nc.compile()
    return nc


def build_tile_kernel(nc, tc, xT, qwT, kwT, vwT, owT, cosq, sinqr, cosk,
                      sinkr, trimask, out):
    from contextlib import ExitStack

    ctx = ExitStack()
    with ctx:
        const = ctx.enter_context(tc.tile_pool(name="const", bufs=1))
        resid = ctx.enter_context(tc.tile_pool(name="resid", bufs=1))
        rtmp = ctx.enter_context(tc.tile_pool(name="rtmp", bufs=1))

        # [128,128] of 1/G: ones-matmul over a probs tile gives the softmax
        # denominator broadcast on all 128 partitions; over |w| it gives the
        # group-mean quant scale broadcast likewise.  (1/G exact in bf16.)
        ones = const.tile([128, 128], BF16)
        nc.gpsimd.memset(ones, 1.0 / G)
        ones1 = const.tile([128, 128], BF16)
        nc.gpsimd.memset(ones1, 1.0)
        warm_src = const.tile([128, 512], BF16)
        nc.gpsimd.memset(warm_src, 0.0)
        tri_sb = const.tile([128, 128], BF16)

        cosq_sb = const.tile([128, T], F16)
        sinqr_sb = const.tile([128, T], F16)
        cosk_sb = const.tile([128, T], F16)
        sinkr_sb = const.tile([128, T], F16)

        # residents
        KTl = resid.tile([128, T], BF16)            # roped k^T  [kd, t]
        QTl = resid.tile([128, HL, T], BF16)        # roped q^T  [dh, h, t]
        Vl = resid.tile([128, NKT, HD], BF16)       # v row-major [t, kt, vd]
        VTs = resid.tile([128, T], BF16)            # v^T staging [vd, t]
        OT = resid.tile([128, HL, T], BF16)         # attn out^T [dh, h, q]

        # ---------------- input DMA (priority order) --------------------
        nc.sync.dma_start(tri_sb, trimask)
        with tc.tile_pool(name="wstage", bufs=1) as wst, \
             tc.tile_pool(name="xstage", bufs=1) as xst:
            kw_sb = wst.tile([128, DT, HD], BF16)
            vw_sb = wst.tile([128, DT, HD], BF16)
            qw_sb = wst.tile([128, DT, HL * HD], BF16)
            ow_sb = wst.tile([128, HL, D], BF16)
            XT = xst.tile([128, DT, T], BF16)

            nc.sync.dma_start(kw_sb, kwT.rearrange("(dt p) r -> p dt r", p=128))
            nc.sync.dma_start(vw_sb, vwT.rearrange("(dt p) r -> p dt r", p=128))
            nc.sync.dma_start(qw_sb, qwT.rearrange("(dt p) r -> p dt r", p=128))
            xsrc = xT.rearrange("(dt p) t -> p dt t", p=128)
            for dt in range(DT):
                nc.sync.dma_start(XT[:, dt, :], xsrc[:, dt, :])
            nc.sync.dma_start(cosq_sb, cosq)
            nc.sync.dma_start(sinqr_sb, sinqr)
            nc.sync.dma_start(cosk_sb, cosk)
            nc.sync.dma_start(sinkr_sb, sinkr)
            nc.sync.dma_start(ow_sb, owT.rearrange("(j p) d -> p j d", p=128))

            run_compute(nc, tc, ctx, const, resid, rtmp, ones, ones1,
                        warm_src, tri_sb, cosq_sb, sinqr_sb, cosk_sb,
                        sinkr_sb, kw_sb, vw_sb, qw_sb, ow_sb, XT, KTl, QTl,
                        Vl, VTs, OT, out)


def run_compute(nc, tc, ctx, const, resid, rtmp, ones, ones1, warm_src,
                tri_sb, cosq_sb, sinqr_sb, cosk_sb, sinkr_sb, kw_sb, vw_sb,
                qw_sb, ow_sb, XT, KTl, QTl, Vl, VTs, OT, out):

    def rope_evac(ps, cos_sb, sinr_sb, col0, w, out_ap):
        """out = ps*cos + rot(ps)*sinr (bf16).  ACT evacuates PSUM (fast
        port), DVE multiplies, gpsimd adds.  sinr tables arrive pre-rolled
        64 partitions for base-partition legality."""
        pse = rtmp.tile([128, w], F32, tag="pse", bufs=3)
        nc.scalar.copy(pse, ps)
        t1 = rtmp.tile([128, w], F32, tag="t1", bufs=2)
        t2 = rtmp.tile([128, w], F32, tag="t2", bufs=2)
        cs = cos_sb[:, col0:col0 + w]
        sr = sinr_sb[:, col0:col0 + w]
        nc.vector.tensor_tensor(t1, pse, cs, op=MULT)
        nc.vector.tensor_tensor(t2[0:64, :], pse[64:128, :], sr[64:128, :],
                                op=MULT)
        nc.vector.tensor_tensor(t2[64:128, :], pse[0:64, :], sr[0:64, :],
                                op=MULT)
        nc.gpsimd.tensor_tensor(out_ap, t1, t2, op=ADD)

    # ============ phase A: quant small weights + K projection ============
    with tc.tile_pool(name="psA", bufs=1, space="PSUM") as psA, \
         tc.tile_pool(name="qtmp", bufs=1) as qtmp:

        def warm(n):
            """Dummy matmuls: keep the PE HAM clock gate open while the
            stream is DMA-paced.  Output never read."""
            for _ in range(n):
                wps = psA.tile([128, 512], F32, tag="warm", bufs=1)
                nc.tensor.matmul(wps, ones, warm_src, start=True, stop=True)

        def quant(w_sb, ncols, tag):
            """1-bit quantize a [128, ncols] din-major slab in place.
            Partition dim = one full quant group (G=128)."""
            ab = qtmp.tile([128, 512], BF16, tag="ab", bufs=2)
            for c0 in range(0, ncols, 512):
                w = min(512, ncols - c0)
                src = w_sb[:, c0:c0 + w]
                nc.scalar.activation(ab[:, 0:w], src, Abs)
                sc = psA.tile([128, 512], F32, tag="scale", bufs=2)
                nc.tensor.matmul(sc[:, 0:w], ones, ab[:, 0:w], start=True,
                                 stop=True)
                nc.scalar.sign(src, src)
                nc.vector.tensor_tensor(src, src, sc[:, 0:w], op=MULT)

        # kv/v/q weight quant (chains overlap the weight DMAs)
        warm(8)
        quant(kw_sb.rearrange("p dt r -> p (dt r)"), DT * HD, "kw")
        quant(vw_sb.rearrange("p dt r -> p (dt r)"), DT * HD, "vw")
        warm(12)
        qflat = qw_sb.rearrange("p dt r -> p (dt r)")
        quant(qflat, DT * HL * HD, "qw")
        warm(20)

        # K projection, dt-outer (paced by XT chunk arrival; dummy matmuls
        # fill the DMA slack to keep the clock gate open)
        psK = psA.tile([128, T], F32, tag="psK")
        for dt in range(DT):
            for cc in range(NQC):
                nc.tensor.matmul(psK[:, 512 * cc:512 * (cc + 1)],
                                 kw_sb[:, dt, :],
                                 XT[:, dt, 512 * cc:512 * (cc + 1)],
                                 start=(dt == 0), stop=(dt == DT - 1))
            warm(2)
        for cc in range(NQC):
            rope_evac(psK[:, 512 * cc:512 * (cc + 1)], cosk_sb, sinkr_sb,
                      512 * cc, 512, KTl[:, 512 * cc:512 * (cc + 1)])

    # ============ phase B: Q heads + V projection + ow quant =============
    with tc.tile_pool(name="psB", bufs=1, space="PSUM") as psB, \
         tc.tile_pool(name="qtmpB", bufs=1) as qtmpB:

        def quantB(src):
            ab = qtmpB.tile([128, 512], BF16, tag="ab", bufs=2)
            nc.scalar.activation(ab, src, Abs)
            sc = psB.tile([128, 512], F32, tag="scale", bufs=2)
            nc.tensor.matmul(sc, ones, ab, start=True, stop=True)
            nc.scalar.sign(src, src)
            nc.vector.tensor_tensor(src, src, sc, op=MULT)

        ow_chunks = [(j, c0) for j in range(HL) for c0 in range(0, D, 512)]
        owi = 0

        def ow_quant_step(n):
            nonlocal owi
            for _ in range(n):
                if owi >= len(ow_chunks):
                    return
                j, c0 = ow_chunks[owi]
                owi += 1
                quantB(ow_sb[:, j, c0:c0 + 512])

        # Q streams (cc-outer: 16 back-to-back 512-col matmuls per chunk)
        for h in range(HL):
            for cc in range(NQC):
                ps = psB.tile([128, 512], F32, tag="acc", bufs=3)
                for dt in range(DT):
                    nc.tensor.matmul(ps, qw_sb[:, dt, 128 * h:128 * (h + 1)],
                                     XT[:, dt, 512 * cc:512 * (cc + 1)],
                                     start=(dt == 0), stop=(dt == DT - 1))
                rope_evac(ps, cosq_sb, sinqr_sb, 512 * cc, 512,
                          QTl[:, h, 512 * cc:512 * (cc + 1)])
                if h >= 1:
                    ow_quant_step(2)

        # V projection -> V^T, then XBAR-transpose to row-major V tiles
        for cc in range(NQC):
            ps = psB.tile([128, 512], F32, tag="acc", bufs=3)
            for dt in range(DT):
                nc.tensor.matmul(ps, vw_sb[:, dt, :],
                                 XT[:, dt, 512 * cc:512 * (cc + 1)],
                                 start=(dt == 0), stop=(dt == DT - 1))
            nc.scalar.copy(VTs[:, 512 * cc:512 * (cc + 1)], ps)
            nc.sync.dma_start_transpose(Vl[:, 4 * cc:4 * (cc + 1), :],
                                        VTs[:, 512 * cc:512 * (cc + 1)])
            ow_quant_step(1)
        ow_quant_step(99)

    # ===================== phase C: attention ===========================
    with tc.tile_pool(name="psC", bufs=1, space="PSUM") as psC, \
         tc.tile_pool(name="apool", bufs=1) as apool:
        tri_b = tri_sb.rearrange("p (o c) -> p o c", o=1)

        for hp in range(HL // 2):
            h0 = 2 * hp
            for m in range(NQC):
                q0 = 512 * m
                nkt = 4 * (m + 1)
                po = psC.tile([128, 2, 512], F32, tag="po")
                pd = psC.tile([128, 2, 512], F32, tag="pd")
                pend = None

                def acc(kt, qoff, pt):
                    first, last = kt == 0, kt == nkt - 1
                    for hh in range(2):
                        nc.tensor.matmul(pd[:, hh, qoff:], ones1,
                                         pt[:, hh, qoff:],
                                         start=first, stop=last)
                        nc.tensor.matmul(po[:, hh, qoff:], Vl[:, kt, :],
                                         pt[:, hh, qoff:],
                                         start=first, stop=last)

                for kt in range(nkt):
                    kc = 128 * kt
                    dj = kt - 4 * m
                    qoff = 128 * dj if dj >= 0 else 0
                    st = psC.tile([128, 2, 512], F32, tag="st", bufs=2)
                    for hh in range(2):
                        nc.tensor.matmul(st[:, hh, qoff:],
                                         KTl[:, kc:kc + 128],
                                         QTl[:, h0 + hh, q0 + qoff:q0 + 512],
                                         start=True, stop=True)
                    pt = apool.tile([128, 2, 512], BF16, tag="pt", bufs=3)
                    nc.scalar.activation(pt[:, :, qoff:], st[:, :, qoff:], Exp)
                    if dj >= 0:
                        ptv = pt[:, :, qoff:qoff + 128]
                        nc.gpsimd.tensor_tensor(ptv, ptv, _bcast(tri_b, ptv),
                                                op=MULT)
                    if pend is not None:
                        acc(*pend)
                    pend = (kt, qoff, pt)
                acc(*pend)

                rq = apool.tile([128, 2, 512], F32, tag="rq", bufs=2)
                nc.vector.reciprocal_approx_fast(rq, pd)
                nc.vector.tensor_tensor(OT[:, h0:h0 + 2, q0:q0 + 512], po, rq,
                                        op=MULT)

    # ==================== phase D: output projection ====================
    with tc.tile_pool(name="psD", bufs=1, space="PSUM") as psD, \
         tc.tile_pool(name="opool", bufs=1) as opool:
        for qt in range(NKT):
            op = psD.tile([128, D], F32, tag="op", bufs=2)
            for cc in range(NQC):
                for ht in range(HL):
                    nc.tensor.matmul(op[:, 512 * cc:512 * (cc + 1)],
                                     OT[:, ht, 128 * qt:128 * (qt + 1)],
                                     ow_sb[:, ht, 512 * cc:512 * (cc + 1)],
                                     start=(ht == 0), stop=(ht == HL - 1))
            osb = opool.tile([128, D], BF16, tag="osb", bufs=3)
            for cc in range(NQC):
                chunk = slice(512 * cc, 512 * (cc + 1))
                if cc % 2 == 0:
                    nc.vector.tensor_copy(osb[:, chunk], op[:, chunk])
                else:
                    nc.scalar.copy(osb[:, chunk], op[:, chunk])
            nc.sync.dma_start(out[128 * qt:128 * (qt + 1), :], osb)


# ---------------------------------------------------------------------------
# host side
# ---------------------------------------------------------------------------
_CACHE = {}


def _tables():
    inv = 1.0 / (THETA ** (np.arange(0, HD, 2, dtype=np.float64) / HD))
    t = np.arange(T, dtype=np.float64)
    fr = np.outer(t, inv)                      # [T, 64]
    emb = np.concatenate([fr, fr], axis=1)     # [T, 128]
    cosT = np.cos(emb).T                       # [128, T] float64
    sinT = np.sin(emb).T
    sinr = np.empty_like(sinT)
    sinr[0:64] = -sinT[0:64]
    sinr[64:128] = sinT[64:128]
    # rolled by 64 partitions: kernel reads sr[64:128] for out[0:64] etc.
    sinr = np.roll(sinr, 64, axis=0)
    return cosT, sinr


def make_in_maps(hidden, q_w, k_w, v_w, o_w):
    cosT, sinr = _tables()
    f16 = np.float16
    bf = ml_dtypes.bfloat16
    cq = np.ascontiguousarray(cosT * ALPHA_Q).astype(f16)
    sq = np.ascontiguousarray(sinr * ALPHA_Q).astype(f16)
    ck = np.ascontiguousarray(cosT).astype(f16)
    sk = np.ascontiguousarray(sinr).astype(f16)
    tri = (np.arange(128)[:, None] <= np.arange(128)[None, :]).astype(bf)
    in_maps = []
    for c in range(NC):
        b, hg = c // 4, c % 4
        in_maps.append({
            "xT": np.ascontiguousarray(hidden[b].T.astype(bf)),
            "qwT": np.ascontiguousarray(
                q_w[512 * hg:512 * (hg + 1), :].T.astype(bf)),
            "kwT": np.ascontiguousarray(
                k_w[128 * hg:128 * (hg + 1), :].T.astype(bf)),
            "vwT": np.ascontiguousarray(
                v_w[128 * hg:128 * (hg + 1), :].T.astype(bf)),
            "owT": np.ascontiguousarray(
                o_w[:, 512 * hg:512 * (hg + 1)].T.astype(bf)),
            "cosq": cq, "sinqr": sq, "cosk": ck, "sinkr": sk,
            "trimask": tri,
        })
    return in_maps


def kernel(hidden, q_w, k_w, v_w, o_w):
    hidden = np.asarray(hidden, dtype=np.float32)
    q_w = np.ascontiguousarray(np.asarray(q_w, dtype=np.float32))
    k_w = np.ascontiguousarray(np.asarray(k_w, dtype=np.float32))
    v_w = np.ascontiguousarray(np.asarray(v_w, dtype=np.float32))
    o_w = np.ascontiguousarray(np.asarray(o_w, dtype=np.float32))

    if "nc" not in _CACHE:
        _CACHE["nc"] = build_program()
    nc = _CACHE["nc"]

    in_maps = make_in_maps(hidden, q_w, k_w, v_w, o_w)
    from concourse.bass_utils import run_bass_kernel_spmd
    res = run_bass_kernel_spmd(nc, in_maps, core_ids=list(range(NC)))
    out = np.zeros((B, T, D), dtype=np.float32)
    for c in range(NC):
        out[c // 4] += res.results[c]["out"].astype(np.float32)
    return out


if __name__ == "__main__":
    print("building program...")
    nc = build_program()
    print("BUILD OK")


# revision 20
# speedup vs baseline: 1.9869x; 1.0375x over previous
"""Trainium2 Bass kernel for GroupedQueryAttention with 1-bit quantized linears.

Sharding (v2): 8 cores = 2 batches x 4 head-groups (tensor-parallel over
heads).  Core c handles batch b=c//4 and head-group hg=c%4: query heads
4hg..4hg+3, kv head hg, ALL 2048 tokens.  The output projection is computed
over the local 512 attention-output columns only -> each core emits a partial
[T, D] sum; the host adds the 4 partials per batch.  No K/V compute
replication (the v1 baseline recomputed full K/V on 4 cores each).

Per-core FLOPs drop 23.6 -> 17.2 GF and every matmul runs 512 output
columns (single PSUM bank) with contraction 128, emitted as one dense
back-to-back PE stream:

 - The PE HAM clock gate defaults to 1.2 GHz and only reaches 2.4 GHz after
   ~3.4us of continuous busy; idle windows re-throttle.  The v1 kernel ran
   mostly cold.  Here the PE stream is kept dense (software-pipelined
   attention, per-512-chunk projection evacuation, double-buffered PSUM) and
   dummy matmuls fill the DMA-bound first ~30us to hold the gate open.
 - Weights arrive host-pre-transposed (din-major); 1-bit quantization runs
   in that layout: scale = ones-matmul partition-reduce of |w| (broadcast to
   all partitions), then sign (ACT) * scale (DVE) in place.  No DRAM round
   trip, no on-device weight transposes.
 - Attention per (head, 512-query quarter): scores[k,q] via K-tile
   stationary, one strided exp over both heads' PSUM banks, triangular
   dmask on diagonal tiles, softmax denominator via ones-matmul (sum lands
   broadcast on all partitions -> wide fast reciprocal), V accumulation in
   PSUM, normalize on evac.
 - O-projection: OT stationary x quantized owT moving, 4x512-col chunks,
   bf16 partial out streamed to DRAM per 128-row tile.

Program is identical across cores; all per-core variation is input data.
"""

import sys

sys.path.insert(0, "/opt/trn_rl_repo")

import numpy as np
import ml_dtypes

import concourse.bacc as bacc
import concourse.bass as bass
import concourse.mybir as mybir
import concourse.tile as tile

F32 = mybir.dt.float32
F16 = mybir.dt.float16
BF16 = mybir.dt.bfloat16

B, T, D = 2, 2048, 2048
H, HK, HD = 16, 4, 128
G = 128
THETA = 1000000.0
NC = 8
HL = H // 4          # 4 local query heads per core
DT = D // 128        # 16 din tiles
NKT = T // 128       # 16 key tiles
NQC = T // 512       # 4 query quarters

ALPHA_Q = HD ** -0.5

Exp = mybir.ActivationFunctionType.Exp
Abs = mybir.ActivationFunctionType.Abs
MULT = mybir.AluOpType.mult
ADD = mybir.AluOpType.add
ABSMAX = mybir.AluOpType.abs_max


def _bcast(ap_small, like_ap):
    a, b = bass.broadcast_tensor_aps(like_ap, ap_small)
    return b


def build_program():
    nc = bacc.Bacc("TRN2", target_bir_lowering=False, debug=False, num_devices=NC)

    xT = nc.dram_tensor("xT", [D, T], BF16, kind="ExternalInput").ap()
    qwT = nc.dram_tensor("qwT", [D, HL * HD], BF16, kind="ExternalInput").ap()
    kwT = nc.dram_tensor("kwT", [D, HD], BF16, kind="ExternalInput").ap()
    vwT = nc.dram_tensor("vwT", [D, HD], BF16, kind="ExternalInput").ap()
    owT = nc.dram_tensor("owT", [HL * HD, D], BF16, kind="ExternalInput").ap()
    cosq = nc.dram_tensor("cosq", [HD, T], F16, kind="ExternalInput").ap()
    sinqr = nc.dram_tensor("sinqr", [HD, T], F16, kind="ExternalInput").ap()
    cosk = nc.dram_tensor("cosk", [HD, T], F16, kind="ExternalInput").ap()
    sinkr = nc.dram_tensor("sinkr", [HD, T], F16, kind="ExternalInput").ap()
    trimask = nc.dram_tensor("trimask", [128, 128], BF16, kind="ExternalInput").ap()
    out = nc.dram_tensor("out", [T, D], BF16, kind="ExternalOutput").ap()

    with tile.TileContext(nc) as tc:
        build_tile_kernel(nc, tc, xT, qwT, kwT, vwT, owT, cosq, sinqr, cosk,
                          sinkr, trimask, out)
    nc.compile()
    return nc


def build_tile_kernel(nc, tc, xT, qwT, kwT, vwT, owT, cosq, sinqr, cosk,
                      sinkr, trimask, out):
    from contextlib import ExitStack

    ctx = ExitStack()
    with ctx:
        const = ctx.enter_context(tc.tile_pool(name="const", bufs=1))
        resid = ctx.enter_context(tc.tile_pool(name="resid", bufs=1))
        rtmp = ctx.enter_context(tc.tile_pool(name="rtmp", bufs=1))

        # [128,128] of 1/G: ones-matmul over a probs tile gives the softmax
        # denominator broadcast on all 128 partitions; over |w| it gives the
        # group-mean quant scale broadcast likewise.  (1/G exact in bf16.)
        ones = const.tile([128, 128], BF16)
        nc.gpsimd.memset(ones, 1.0 / G)
        ones1 = const.tile([128, 128], BF16)
        nc.gpsimd.memset(ones1, 1.0)
        warm_src = const.tile([128, 512], BF16)
        nc.gpsimd.memset(warm_src, 0.0)
        tri_sb = const.tile([128, 128], BF16)

        cosq_sb = const.tile([128, T], F16)
        sinqr_sb = const.tile([128, T], F16)
        cosk_sb = const.tile([128, T], F16)
        sinkr_sb = const.tile([128, T], F16)

        # residents
        KTl = resid.tile([128, T], BF16)            # roped k^T  [kd, t]
        QTl = resid.tile([128, HL, T], BF16)        # roped q^T  [dh, h, t]
        Vl = resid.tile([128, NKT, HD], BF16)       # v row-major [t, kt, vd]
        VTs = resid.tile([128, T], BF16)            # v^T staging [vd, t]
        OT = resid.tile([128, HL, T], BF16)         # attn out^T [dh, h, q]

        # ---------------- input DMA (priority order) --------------------
        nc.sync.dma_start(tri_sb, trimask)
        with tc.tile_pool(name="wstage", bufs=1) as wst, \
             tc.tile_pool(name="xstage", bufs=1) as xst:
            kw_sb = wst.tile([128, DT, HD], BF16)
            vw_sb = wst.tile([128, DT, HD], BF16)
            qw_sb = wst.tile([128, DT, HL * HD], BF16)
            ow_sb = wst.tile([128, HL, D], BF16)
            XT = xst.tile([128, DT, T], BF16)

            nc.sync.dma_start(kw_sb, kwT.rearrange("(dt p) r -> p dt r", p=128))
            nc.sync.dma_start(vw_sb, vwT.rearrange("(dt p) r -> p dt r", p=128))
            nc.sync.dma_start(qw_sb, qwT.rearrange("(dt p) r -> p dt r", p=128))
            xsrc = xT.rearrange("(dt p) t -> p dt t", p=128)
            for dt in range(DT):
                nc.sync.dma_start(XT[:, dt, :], xsrc[:, dt, :])
            nc.sync.dma_start(cosk_sb, cosk)
            nc.sync.dma_start(sinkr_sb, sinkr)
            nc.sync.dma_start(cosq_sb, cosq)
            nc.sync.dma_start(sinqr_sb, sinqr)
            nc.sync.dma_start(ow_sb, owT.rearrange("(j p) d -> p j d", p=128))

            run_compute(nc, tc, ctx, const, resid, rtmp, ones, ones1,
                        warm_src, tri_sb, cosq_sb, sinqr_sb, cosk_sb,
                        sinkr_sb, kw_sb, vw_sb, qw_sb, ow_sb, XT, KTl, QTl,
                        Vl, VTs, OT, out)


def run_compute(nc, tc, ctx, const, resid, rtmp, ones, ones1, warm_src,
                tri_sb, cosq_sb, sinqr_sb, cosk_sb, sinkr_sb, kw_sb, vw_sb,
                qw_sb, ow_sb, XT, KTl, QTl, Vl, VTs, OT, out):

    def rope_evac(ps, cos_sb, sinr_sb, col0, w, out_ap):
        """out = ps*cos + rot(ps)*sinr (bf16).  ACT evacuates PSUM (fast
        port), DVE multiplies, gpsimd adds.  sinr tables arrive pre-rolled
        64 partitions for base-partition legality."""
        pse = rtmp.tile([128, w], F32, tag="pse", bufs=3)
        nc.scalar.copy(pse, ps)
        t1 = rtmp.tile([128, w], F32, tag="t1", bufs=2)
        t2 = rtmp.tile([128, w], F32, tag="t2", bufs=2)
        cs = cos_sb[:, col0:col0 + w]
        sr = sinr_sb[:, col0:col0 + w]
        nc.vector.tensor_tensor(t1, pse, cs, op=MULT)
        nc.vector.tensor_tensor(t2[0:64, :], pse[64:128, :], sr[64:128, :],
                                op=MULT)
        nc.vector.tensor_tensor(t2[64:128, :], pse[0:64, :], sr[0:64, :],
                                op=MULT)
        nc.gpsimd.tensor_tensor(out_ap, t1, t2, op=ADD)

    # ============ phase A: quant small weights + K projection ============
    with tc.tile_pool(name="psA", bufs=1, space="PSUM") as psA, \
         tc.tile_pool(name="qtmp", bufs=1) as qtmp:

        def warm(n):
            """Dummy matmuls: keep the PE HAM clock gate open while the
            stream is DMA-paced.  Output never read."""
            for _ in range(n):
                wps = psA.tile([128, 512], F32, tag="warm", bufs=1)
                nc.tensor.matmul(wps, ones, warm_src, start=True, stop=True)

        def quant(w_sb, ncols, tag):
            """1-bit quantize a [128, ncols] din-major slab in place.
            Partition dim = one full quant group (G=128)."""
            ab = qtmp.tile([128, 512], BF16, tag="ab", bufs=2)
            for c0 in range(0, ncols, 512):
                w = min(512, ncols - c0)
                src = w_sb[:, c0:c0 + w]
                nc.scalar.activation(ab[:, 0:w], src, Abs)
                sc = psA.tile([128, 512], F32, tag="scale", bufs=2)
                nc.tensor.matmul(sc[:, 0:w], ones, ab[:, 0:w], start=True,
                                 stop=True)
                nc.scalar.sign(src, src)
                nc.vector.tensor_tensor(src, src, sc[:, 0:w], op=MULT)

        # k/v weight quant (chains overlap the weight DMAs)
        warm(8)
        quant(kw_sb.rearrange("p dt r -> p (dt r)"), DT * HD, "kw")
        quant(vw_sb.rearrange("p dt r -> p (dt r)"), DT * HD, "vw")
        warm(4)

        # K projection, dt-outer (paced by XT chunk arrival); the qw quant
        # chains ride along as PE fillers for the DMA slack, which also
        # keeps the clock gate open
        qflat = qw_sb.rearrange("p dt r -> p (dt r)")
        psK = psA.tile([128, T], F32, tag="psK")
        for dt in range(DT):
            for cc in range(NQC):
                nc.tensor.matmul(psK[:, 512 * cc:512 * (cc + 1)],
                                 kw_sb[:, dt, :],
                                 XT[:, dt, 512 * cc:512 * (cc + 1)],
                                 start=(dt == 0), stop=(dt == DT - 1))
            quant(qflat[:, 512 * dt:512 * (dt + 1)], 512, "qw")
            if dt < 10:
                warm(1)
        for cc in range(NQC):
            rope_evac(psK[:, 512 * cc:512 * (cc + 1)], cosk_sb, sinkr_sb,
                      512 * cc, 512, KTl[:, 512 * cc:512 * (cc + 1)])

    # ============ phase B: Q heads + V projection + ow quant =============
    with tc.tile_pool(name="psB", bufs=1, space="PSUM") as psB, \
         tc.tile_pool(name="qtmpB", bufs=1) as qtmpB:

        def quantB(src):
            ab = qtmpB.tile([128, 512], BF16, tag="ab", bufs=2)
            nc.scalar.activation(ab, src, Abs)
            sc = psB.tile([128, 512], F32, tag="scale", bufs=2)
            nc.tensor.matmul(sc, ones, ab, start=True, stop=True)
            nc.scalar.sign(src, src)
            nc.vector.tensor_tensor(src, src, sc, op=MULT)

        ow_chunks = [(j, c0) for j in range(HL) for c0 in range(0, D, 512)]
        owi = 0

        def ow_quant_step(n):
            nonlocal owi
            for _ in range(n):
                if owi >= len(ow_chunks):
                    return
                j, c0 = ow_chunks[owi]
                owi += 1
                quantB(ow_sb[:, j, c0:c0 + 512])

        # Q streams (cc-outer: 16 back-to-back 512-col matmuls per chunk)
        for h in range(HL):
            for cc in range(NQC):
                ps = psB.tile([128, 512], F32, tag="acc", bufs=3)
                for dt in range(DT):
                    nc.tensor.matmul(ps, qw_sb[:, dt, 128 * h:128 * (h + 1)],
                                     XT[:, dt, 512 * cc:512 * (cc + 1)],
                                     start=(dt == 0), stop=(dt == DT - 1))
                rope_evac(ps, cosq_sb, sinqr_sb, 512 * cc, 512,
                          QTl[:, h, 512 * cc:512 * (cc + 1)])
                if h >= 1:
                    ow_quant_step(2)

        # V projection -> V^T, then XBAR-transpose to row-major V tiles
        for cc in range(NQC):
            ps = psB.tile([128, 512], F32, tag="acc", bufs=3)
            for dt in range(DT):
                nc.tensor.matmul(ps, vw_sb[:, dt, :],
                                 XT[:, dt, 512 * cc:512 * (cc + 1)],
                                 start=(dt == 0), stop=(dt == DT - 1))
            nc.scalar.copy(VTs[:, 512 * cc:512 * (cc + 1)], ps)
            nc.sync.dma_start_transpose(Vl[:, 4 * cc:4 * (cc + 1), :],
                                        VTs[:, 512 * cc:512 * (cc + 1)])
            ow_quant_step(1)
        ow_quant_step(99)

    # ===================== phase C: attention ===========================
    with tc.tile_pool(name="psC", bufs=1, space="PSUM") as psC, \
         tc.tile_pool(name="apool", bufs=1) as apool:
        for hp in range(HL // 2):
            h0 = 2 * hp
            for m in range(NQC):
                q0 = 512 * m
                nkt = 4 * (m + 1)
                po = psC.tile([128, 2, 512], F32, tag="po")
                pd = psC.tile([128, 2, 512], F32, tag="pd")
                pend = None

                def acc(kt, qoff, pt):
                    first, last = kt == 0, kt == nkt - 1
                    for hh in range(2):
                        nc.tensor.matmul(pd[:, hh, qoff:], ones1,
                                         pt[:, hh, qoff:],
                                         start=first, stop=last)
                        nc.tensor.matmul(po[:, hh, qoff:], Vl[:, kt, :],
                                         pt[:, hh, qoff:],
                                         start=first, stop=last)

                for kt in range(nkt):
                    kc = 128 * kt
                    dj = kt - 4 * m
                    qoff = 128 * dj if dj >= 0 else 0
                    st = psC.tile([128, 2, 512], F32, tag="st", bufs=2)
                    for hh in range(2):
                        nc.tensor.matmul(st[:, hh, qoff:],
                                         KTl[:, kc:kc + 128],
                                         QTl[:, h0 + hh, q0 + qoff:q0 + 512],
                                         start=True, stop=True)
                    pt = apool.tile([128, 2, 512], BF16, tag="pt", bufs=3)
                    for hh in range(2):
                        nc.scalar.activation(pt[:, hh, qoff:],
                                             st[:, hh, qoff:], Exp)
                        if dj >= 0:
                            blk = slice(qoff, qoff + 128)
                            nc.vector.tensor_tensor(pt[:, hh, blk],
                                                    pt[:, hh, blk], tri_sb,
                                                    op=MULT)
                    if pend is not None:
                        acc(*pend)
                    pend = (kt, qoff, pt)
                acc(*pend)

                rq = apool.tile([128, 2, 512], F32, tag="rq", bufs=2)
                nc.vector.reciprocal_approx_fast(rq, pd)
                nc.vector.tensor_tensor(OT[:, h0:h0 + 2, q0:q0 + 512], po, rq,
                                        op=MULT)

    # ==================== phase D: output projection ====================
    with tc.tile_pool(name="psD", bufs=1, space="PSUM") as psD, \
         tc.tile_pool(name="opool", bufs=1) as opool:
        for qt in range(NKT):
            op = psD.tile([128, D], F32, tag="op", bufs=2)
            for cc in range(NQC):
                for ht in range(HL):
                    nc.tensor.matmul(op[:, 512 * cc:512 * (cc + 1)],
                                     OT[:, ht, 128 * qt:128 * (qt + 1)],
                                     ow_sb[:, ht, 512 * cc:512 * (cc + 1)],
                                     start=(ht == 0), stop=(ht == HL - 1))
            osb = opool.tile([128, D], BF16, tag="osb", bufs=3)
            for cc in range(NQC):
                chunk = slice(512 * cc, 512 * (cc + 1))
                nc.vector.tensor_copy(osb[:, chunk], op[:, chunk])
            # alternate DGE queues (SP / ACT) so the 8MB output never
            # backlogs one ring and the drain tail stays short
            eng = nc.sync if qt % 2 == 0 else nc.scalar
            eng.dma_start(out[128 * qt:128 * (qt + 1), :], osb)


# ---------------------------------------------------------------------------
# host side
# ---------------------------------------------------------------------------
_CACHE = {}


def _tables():
    inv = 1.0 / (THETA ** (np.arange(0, HD, 2, dtype=np.float64) / HD))
    t = np.arange(T, dtype=np.float64)
    fr = np.outer(t, inv)                      # [T, 64]
    emb = np.concatenate([fr, fr], axis=1)     # [T, 128]
    cosT = np.cos(emb).T                       # [128, T] float64
    sinT = np.sin(emb).T
    sinr = np.empty_like(sinT)
    sinr[0:64] = -sinT[0:64]
    sinr[64:128] = sinT[64:128]
    # rolled by 64 partitions: kernel reads sr[64:128] for out[0:64] etc.
    sinr = np.roll(sinr, 64, axis=0)
    return cosT, sinr


def make_in_maps(hidden, q_w, k_w, v_w, o_w):
    cosT, sinr = _tables()
    f16 = np.float16
    bf = ml_dtypes.bfloat16
    cq = np.ascontiguousarray(cosT * ALPHA_Q).astype(f16)
    sq = np.ascontiguousarray(sinr * ALPHA_Q).astype(f16)
    ck = np.ascontiguousarray(cosT).astype(f16)
    sk = np.ascontiguousarray(sinr).astype(f16)
    tri = (np.arange(128)[:, None] <= np.arange(128)[None, :]).astype(bf)
    in_maps = []
    for c in range(NC):
        b, hg = c // 4, c % 4
        in_maps.append({
            "xT": np.ascontiguousarray(hidden[b].T.astype(bf)),
            "qwT": np.ascontiguousarray(
                q_w[512 * hg:512 * (hg + 1), :].T.astype(bf)),
            "kwT": np.ascontiguousarray(
                k_w[128 * hg:128 * (hg + 1), :].T.astype(bf)),
            "vwT": np.ascontiguousarray(
                v_w[128 * hg:128 * (hg + 1), :].T.astype(bf)),
            "owT": np.ascontiguousarray(
                o_w[:, 512 * hg:512 * (hg + 1)].T.astype(bf)),
            "cosq": cq, "sinqr": sq, "cosk": ck, "sinkr": sk,
            "trimask": tri,
        })
    return in_maps


def kernel(hidden, q_w, k_w, v_w, o_w):
    hidden = np.asarray(hidden, dtype=np.float32)
    q_w = np.ascontiguousarray(np.asarray(q_w, dtype=np.float32))
    k_w = np.ascontiguousarray(np.asarray(k_w, dtype=np.float32))
    v_w = np.ascontiguousarray(np.asarray(v_w, dtype=np.float32))
    o_w = np.ascontiguousarray(np.asarray(o_w, dtype=np.float32))

    if "nc" not in _CACHE:
        _CACHE["nc"] = build_program()
    nc = _CACHE["nc"]

    in_maps = make_in_maps(hidden, q_w, k_w, v_w, o_w)
    from concourse.bass_utils import run_bass_kernel_spmd
    res = run_bass_kernel_spmd(nc, in_maps, core_ids=list(range(NC)))
    out = np.zeros((B, T, D), dtype=np.float32)
    for c in range(NC):
        out[c // 4] += res.results[c]["out"].astype(np.float32)
    return out


if __name__ == "__main__":
    print("building program...")
    nc = build_program()
    print("BUILD OK")


# revision 23
# speedup vs baseline: 2.0091x; 1.0111x over previous
"""Trainium2 Bass kernel for GroupedQueryAttention with 1-bit quantized linears.

Sharding (v2): 8 cores = 2 batches x 4 head-groups (tensor-parallel over
heads).  Core c handles batch b=c//4 and head-group hg=c%4: query heads
4hg..4hg+3, kv head hg, ALL 2048 tokens.  The output projection is computed
over the local 512 attention-output columns only -> each core emits a partial
[T, D] sum; the host adds the 4 partials per batch.  No K/V compute
replication (the v1 baseline recomputed full K/V on 4 cores each).

Per-core FLOPs drop 23.6 -> 17.2 GF and every matmul runs 512 output
columns (single PSUM bank) with contraction 128, emitted as one dense
back-to-back PE stream:

 - The PE HAM clock gate defaults to 1.2 GHz and only reaches 2.4 GHz after
   ~3.4us of continuous busy; idle windows re-throttle.  The v1 kernel ran
   mostly cold.  Here the PE stream is kept dense (software-pipelined
   attention, per-512-chunk projection evacuation, double-buffered PSUM) and
   dummy matmuls fill the DMA-bound first ~30us to hold the gate open.
 - Weights arrive host-pre-transposed (din-major); 1-bit quantization runs
   in that layout: scale = ones-matmul partition-reduce of |w| (broadcast to
   all partitions), then sign (ACT) * scale (DVE) in place.  No DRAM round
   trip, no on-device weight transposes.
 - Attention per (head, 512-query quarter): scores[k,q] via K-tile
   stationary, one strided exp over both heads' PSUM banks, triangular
   dmask on diagonal tiles, softmax denominator via ones-matmul (sum lands
   broadcast on all partitions -> wide fast reciprocal), V accumulation in
   PSUM, normalize on evac.
 - O-projection: OT stationary x quantized owT moving, 4x512-col chunks,
   bf16 partial out streamed to DRAM per 128-row tile.

Program is identical across cores; all per-core variation is input data.
"""

import sys

sys.path.insert(0, "/opt/trn_rl_repo")

import numpy as np
import ml_dtypes

import concourse.bacc as bacc
import concourse.bass as bass
import concourse.mybir as mybir
import concourse.tile as tile

F32 = mybir.dt.float32
F16 = mybir.dt.float16
BF16 = mybir.dt.bfloat16

B, T, D = 2, 2048, 2048
H, HK, HD = 16, 4, 128
G = 128
THETA = 1000000.0
NC = 8
HL = H // 4          # 4 local query heads per core
DT = D // 128        # 16 din tiles
NKT = T // 128       # 16 key tiles
NQC = T // 512       # 4 query quarters

ALPHA_Q = HD ** -0.5

Exp = mybir.ActivationFunctionType.Exp
Abs = mybir.ActivationFunctionType.Abs
MULT = mybir.AluOpType.mult
ADD = mybir.AluOpType.add
ABSMAX = mybir.AluOpType.abs_max


def _bcast(ap_small, like_ap):
    a, b = bass.broadcast_tensor_aps(like_ap, ap_small)
    return b


def build_program():
    nc = bacc.Bacc("TRN2", target_bir_lowering=False, debug=False, num_devices=NC)

    xT = nc.dram_tensor("xT", [D, T], BF16, kind="ExternalInput").ap()
    qwT = nc.dram_tensor("qwT", [D, HL * HD], BF16, kind="ExternalInput").ap()
    kwT = nc.dram_tensor("kwT", [D, HD], BF16, kind="ExternalInput").ap()
    vwT = nc.dram_tensor("vwT", [D, HD], BF16, kind="ExternalInput").ap()
    owT = nc.dram_tensor("owT", [HL * HD, D], BF16, kind="ExternalInput").ap()
    cosq = nc.dram_tensor("cosq", [HD, T], F16, kind="ExternalInput").ap()
    sinqr = nc.dram_tensor("sinqr", [HD, T], F16, kind="ExternalInput").ap()
    cosk = nc.dram_tensor("cosk", [HD, T], F16, kind="ExternalInput").ap()
    sinkr = nc.dram_tensor("sinkr", [HD, T], F16, kind="ExternalInput").ap()
    trimask = nc.dram_tensor("trimask", [128, 128], BF16, kind="ExternalInput").ap()
    out = nc.dram_tensor("out", [T, D], BF16, kind="ExternalOutput").ap()

    with tile.TileContext(nc) as tc:
        build_tile_kernel(nc, tc, xT, qwT, kwT, vwT, owT, cosq, sinqr, cosk,
                          sinkr, trimask, out)
    nc.compile()
    return nc


def build_tile_kernel(nc, tc, xT, qwT, kwT, vwT, owT, cosq, sinqr, cosk,
                      sinkr, trimask, out):
    from contextlib import ExitStack

    ctx = ExitStack()
    with ctx:
        const = ctx.enter_context(tc.tile_pool(name="const", bufs=1))
        resid = ctx.enter_context(tc.tile_pool(name="resid", bufs=1))
        rtmp = ctx.enter_context(tc.tile_pool(name="rtmp", bufs=1))

        # [128,128] of 1/G: ones-matmul over a probs tile gives the softmax
        # denominator broadcast on all 128 partitions; over |w| it gives the
        # group-mean quant scale broadcast likewise.  (1/G exact in bf16.)
        ones = const.tile([128, 128], BF16)
        nc.gpsimd.memset(ones, 1.0 / G)
        ones1 = const.tile([128, 128], BF16)
        nc.gpsimd.memset(ones1, 1.0)
        warm_src = const.tile([128, 512], BF16)
        nc.gpsimd.memset(warm_src, 0.0)
        tri_sb = const.tile([128, 128], BF16)

        cosq_sb = const.tile([128, T], F16)
        sinqr_sb = const.tile([128, T], F16)
        cosk_sb = const.tile([128, T], F16)
        sinkr_sb = const.tile([128, T], F16)

        # residents
        KTl = resid.tile([128, T], BF16)            # roped k^T  [kd, t]
        QTl = resid.tile([128, HL, T], BF16)        # roped q^T  [dh, h, t]
        Vl = resid.tile([128, NKT, HD], BF16)       # v row-major [t, kt, vd]
        VTs = resid.tile([128, T], BF16)            # v^T staging [vd, t]
        OT = resid.tile([128, HL, T], BF16)         # attn out^T [dh, h, q]

        # ---------------- input DMA (priority order) --------------------
        nc.sync.dma_start(tri_sb, trimask)
        with tc.tile_pool(name="wstage", bufs=1) as wst, \
             tc.tile_pool(name="xstage", bufs=1) as xst:
            kw_sb = wst.tile([128, DT, HD], BF16)
            vw_sb = wst.tile([128, DT, HD], BF16)
            qw_sb = wst.tile([128, DT, HL * HD], BF16)
            ow_sb = wst.tile([128, HL, D], BF16)
            XT = xst.tile([128, DT, T], BF16)

            nc.sync.dma_start(kw_sb, kwT.rearrange("(dt p) r -> p dt r", p=128))
            nc.sync.dma_start(vw_sb, vwT.rearrange("(dt p) r -> p dt r", p=128))
            nc.sync.dma_start(qw_sb, qwT.rearrange("(dt p) r -> p dt r", p=128))
            xsrc = xT.rearrange("(dt p) t -> p dt t", p=128)
            for dt in range(DT):
                nc.sync.dma_start(XT[:, dt, :], xsrc[:, dt, :])
            nc.sync.dma_start(cosk_sb, cosk)
            nc.sync.dma_start(sinkr_sb, sinkr)
            nc.sync.dma_start(cosq_sb, cosq)
            nc.sync.dma_start(sinqr_sb, sinqr)
            nc.sync.dma_start(ow_sb, owT.rearrange("(j p) d -> p j d", p=128))

            run_compute(nc, tc, ctx, const, resid, rtmp, ones, ones1,
                        warm_src, tri_sb, cosq_sb, sinqr_sb, cosk_sb,
                        sinkr_sb, kw_sb, vw_sb, qw_sb, ow_sb, XT, KTl, QTl,
                        Vl, VTs, OT, out)


def run_compute(nc, tc, ctx, const, resid, rtmp, ones, ones1, warm_src,
                tri_sb, cosq_sb, sinqr_sb, cosk_sb, sinkr_sb, kw_sb, vw_sb,
                qw_sb, ow_sb, XT, KTl, QTl, Vl, VTs, OT, out):

    def rope_evac(ps, cos_sb, sinr_sb, col0, w, out_ap):
        """out = ps*cos + rot(ps)*sinr (bf16).  ACT evacuates PSUM (fast
        port), DVE multiplies, gpsimd adds.  sinr tables arrive pre-rolled
        64 partitions for base-partition legality."""
        pse = rtmp.tile([128, w], F32, tag="pse", bufs=3)
        nc.scalar.copy(pse, ps)
        t1 = rtmp.tile([128, w], F32, tag="t1", bufs=2)
        t2 = rtmp.tile([128, w], F32, tag="t2", bufs=2)
        cs = cos_sb[:, col0:col0 + w]
        sr = sinr_sb[:, col0:col0 + w]
        nc.vector.tensor_tensor(t1, pse, cs, op=MULT)
        nc.vector.tensor_tensor(t2[0:64, :], pse[64:128, :], sr[64:128, :],
                                op=MULT)
        nc.vector.tensor_tensor(t2[64:128, :], pse[0:64, :], sr[0:64, :],
                                op=MULT)
        nc.gpsimd.tensor_tensor(out_ap, t1, t2, op=ADD)

    # ============ phase A: quant small weights + K projection ============
    with tc.tile_pool(name="psA", bufs=1, space="PSUM") as psA, \
         tc.tile_pool(name="qtmp", bufs=1) as qtmp:

        def warm(n):
            """Dummy matmuls: keep the PE HAM clock gate open while the
            stream is DMA-paced.  Output never read."""
            for _ in range(n):
                wps = psA.tile([128, 512], F32, tag="warm", bufs=1)
                nc.tensor.matmul(wps, ones, warm_src, start=True, stop=True)

        def quant(w_sb, ncols, tag):
            """1-bit quantize a [128, ncols] din-major slab in place.
            Partition dim = one full quant group (G=128)."""
            ab = qtmp.tile([128, 512], BF16, tag="ab", bufs=2)
            for c0 in range(0, ncols, 512):
                w = min(512, ncols - c0)
                src = w_sb[:, c0:c0 + w]
                nc.scalar.activation(ab[:, 0:w], src, Abs)
                sc = psA.tile([128, 512], F32, tag="scale", bufs=2)
                nc.tensor.matmul(sc[:, 0:w], ones, ab[:, 0:w], start=True,
                                 stop=True)
                nc.scalar.sign(src, src)
                nc.vector.tensor_tensor(src, src, sc[:, 0:w], op=MULT)

        # k/v weight quant (chains overlap the weight DMAs)
        warm(8)
        quant(kw_sb.rearrange("p dt r -> p (dt r)"), DT * HD, "kw")
        quant(vw_sb.rearrange("p dt r -> p (dt r)"), DT * HD, "vw")
        warm(4)

        # K projection, dt-outer (paced by XT chunk arrival; dummy matmuls
        # fill the DMA slack so the clock gate stays open).  The qw quant
        # matmuls go AFTER the K loop: their ACT abs chains trail the DMA,
        # and a stalled quant matmul in the in-order PE queue would block
        # every K matmul behind it.
        qflat = qw_sb.rearrange("p dt r -> p (dt r)")
        psK = psA.tile([128, T], F32, tag="psK")
        for dt in range(DT):
            for cc in range(NQC):
                nc.tensor.matmul(psK[:, 512 * cc:512 * (cc + 1)],
                                 kw_sb[:, dt, :],
                                 XT[:, dt, 512 * cc:512 * (cc + 1)],
                                 start=(dt == 0), stop=(dt == DT - 1))
            warm(1)
        for dt in range(DT):
            quant(qflat[:, 512 * dt:512 * (dt + 1)], 512, "qw")
        for cc in range(NQC):
            rope_evac(psK[:, 512 * cc:512 * (cc + 1)], cosk_sb, sinkr_sb,
                      512 * cc, 512, KTl[:, 512 * cc:512 * (cc + 1)])

    # ============ phase B: Q heads + V projection + ow quant =============
    with tc.tile_pool(name="psB", bufs=1, space="PSUM") as psB, \
         tc.tile_pool(name="qtmpB", bufs=1) as qtmpB:

        def quantB(src):
            ab = qtmpB.tile([128, 512], BF16, tag="ab", bufs=2)
            nc.scalar.activation(ab, src, Abs)
            sc = psB.tile([128, 512], F32, tag="scale", bufs=2)
            nc.tensor.matmul(sc, ones, ab, start=True, stop=True)
            nc.scalar.sign(src, src)
            nc.vector.tensor_tensor(src, src, sc, op=MULT)

        ow_chunks = [(j, c0) for j in range(HL) for c0 in range(0, D, 512)]
        owi = 0

        def ow_quant_step(n):
            nonlocal owi
            for _ in range(n):
                if owi >= len(ow_chunks):
                    return
                j, c0 = ow_chunks[owi]
                owi += 1
                quantB(ow_sb[:, j, c0:c0 + 512])

        # Q streams (cc-outer: 16 back-to-back 512-col matmuls per chunk)
        for h in range(HL):
            for cc in range(NQC):
                ps = psB.tile([128, 512], F32, tag="acc", bufs=3)
                for dt in range(DT):
                    nc.tensor.matmul(ps, qw_sb[:, dt, 128 * h:128 * (h + 1)],
                                     XT[:, dt, 512 * cc:512 * (cc + 1)],
                                     start=(dt == 0), stop=(dt == DT - 1))
                rope_evac(ps, cosq_sb, sinqr_sb, 512 * cc, 512,
                          QTl[:, h, 512 * cc:512 * (cc + 1)])
                if h >= 1:
                    ow_quant_step(2)

        # V projection -> V^T, then XBAR-transpose to row-major V tiles
        for cc in range(NQC):
            ps = psB.tile([128, 512], F32, tag="acc", bufs=3)
            for dt in range(DT):
                nc.tensor.matmul(ps, vw_sb[:, dt, :],
                                 XT[:, dt, 512 * cc:512 * (cc + 1)],
                                 start=(dt == 0), stop=(dt == DT - 1))
            nc.scalar.copy(VTs[:, 512 * cc:512 * (cc + 1)], ps)
            nc.sync.dma_start_transpose(Vl[:, 4 * cc:4 * (cc + 1), :],
                                        VTs[:, 512 * cc:512 * (cc + 1)])
            ow_quant_step(1)
        ow_quant_step(99)

    # ===================== phase C: attention ===========================
    with tc.tile_pool(name="psC", bufs=1, space="PSUM") as psC, \
         tc.tile_pool(name="apool", bufs=1) as apool:
        for hp in range(HL // 2):
            h0 = 2 * hp
            for m in range(NQC):
                q0 = 512 * m
                nkt = 4 * (m + 1)
                po = psC.tile([128, 2, 512], F32, tag="po")
                pd = psC.tile([128, 2, 512], F32, tag="pd")
                pend = None

                def acc(kt, qoff, pt):
                    first, last = kt == 0, kt == nkt - 1
                    for hh in range(2):
                        nc.tensor.matmul(pd[:, hh, qoff:], ones1,
                                         pt[:, hh, qoff:],
                                         start=first, stop=last)
                        nc.tensor.matmul(po[:, hh, qoff:], Vl[:, kt, :],
                                         pt[:, hh, qoff:],
                                         start=first, stop=last)

                for kt in range(nkt):
                    kc = 128 * kt
                    dj = kt - 4 * m
                    qoff = 128 * dj if dj >= 0 else 0
                    st = psC.tile([128, 2, 512], F32, tag="st", bufs=2)
                    for hh in range(2):
                        nc.tensor.matmul(st[:, hh, qoff:],
                                         KTl[:, kc:kc + 128],
                                         QTl[:, h0 + hh, q0 + qoff:q0 + 512],
                                         start=True, stop=True)
                    pt = apool.tile([128, 2, 512], BF16, tag="pt", bufs=3)
                    nc.scalar.activation(pt[:, :, qoff:], st[:, :, qoff:],
                                         Exp)
                    if dj >= 0:
                        blk = slice(qoff, qoff + 128)
                        for hh in range(2):
                            nc.vector.tensor_tensor(pt[:, hh, blk],
                                                    pt[:, hh, blk], tri_sb,
                                                    op=MULT)
                    if pend is not None:
                        acc(*pend)
                    pend = (kt, qoff, pt)
                acc(*pend)

                rq = apool.tile([128, 2, 512], F32, tag="rq", bufs=2)
                nc.vector.reciprocal_approx_fast(rq, pd)
                nc.vector.tensor_tensor(OT[:, h0:h0 + 2, q0:q0 + 512], po, rq,
                                        op=MULT)

    # ==================== phase D: output projection ====================
    with tc.tile_pool(name="psD", bufs=1, space="PSUM") as psD, \
         tc.tile_pool(name="opool", bufs=1) as opool:
        for qt in range(NKT):
            op = psD.tile([128, D], F32, tag="op", bufs=2)
            for cc in range(NQC):
                for ht in range(HL):
                    nc.tensor.matmul(op[:, 512 * cc:512 * (cc + 1)],
                                     OT[:, ht, 128 * qt:128 * (qt + 1)],
                                     ow_sb[:, ht, 512 * cc:512 * (cc + 1)],
                                     start=(ht == 0), stop=(ht == HL - 1))
            osb = opool.tile([128, D], BF16, tag="osb", bufs=3)
            for cc in range(NQC):
                chunk = slice(512 * cc, 512 * (cc + 1))
                if cc % 2 == 0:
                    nc.vector.tensor_copy(osb[:, chunk], op[:, chunk])
                else:
                    nc.scalar.copy(osb[:, chunk], op[:, chunk])
            # alternate DGE queues (SP / ACT) so the 8MB output never
            # backlogs one ring and the drain tail stays short
            eng = nc.sync if qt % 2 == 0 else nc.scalar
            eng.dma_start(out[128 * qt:128 * (qt + 1), :], osb)


# ---------------------------------------------------------------------------
# host side
# ---------------------------------------------------------------------------
_CACHE = {}


def _tables():
    inv = 1.0 / (THETA ** (np.arange(0, HD, 2, dtype=np.float64) / HD))
    t = np.arange(T, dtype=np.float64)
    fr = np.outer(t, inv)                      # [T, 64]
    emb = np.concatenate([fr, fr], axis=1)     # [T, 128]
    cosT = np.cos(emb).T                       # [128, T] float64
    sinT = np.sin(emb).T
    sinr = np.empty_like(sinT)
    sinr[0:64] = -sinT[0:64]
    sinr[64:128] = sinT[64:128]
    # rolled by 64 partitions: kernel reads sr[64:128] for out[0:64] etc.
    sinr = np.roll(sinr, 64, axis=0)
    return cosT, sinr


def make_in_maps(hidden, q_w, k_w, v_w, o_w):
    cosT, sinr = _tables()
    f16 = np.float16
    bf = ml_dtypes.bfloat16
    cq = np.ascontiguousarray(cosT * ALPHA_Q).astype(f16)
    sq = np.ascontiguousarray(sinr * ALPHA_Q).astype(f16)
    ck = np.ascontiguousarray(cosT).astype(f16)
    sk = np.ascontiguousarray(sinr).astype(f16)
    tri = (np.arange(128)[:, None] <= np.arange(128)[None, :]).astype(bf)
    in_maps = []
    for c in range(NC):
        b, hg = c // 4, c % 4
        in_maps.append({
            "xT": np.ascontiguousarray(hidden[b].T.astype(bf)),
            "qwT": np.ascontiguousarray(
                q_w[512 * hg:512 * (hg + 1), :].T.astype(bf)),
            "kwT": np.ascontiguousarray(
                k_w[128 * hg:128 * (hg + 1), :].T.astype(bf)),
            "vwT": np.ascontiguousarray(
                v_w[128 * hg:128 * (hg + 1), :].T.astype(bf)),
            "owT": np.ascontiguousarray(
                o_w[:, 512 * hg:512 * (hg + 1)].T.astype(bf)),
            "cosq": cq, "sinqr": sq, "cosk": ck, "sinkr": sk,
            "trimask": tri,
        })
    return in_maps


def kernel(hidden, q_w, k_w, v_w, o_w):
    hidden = np.asarray(hidden, dtype=np.float32)
    q_w = np.ascontiguousarray(np.asarray(q_w, dtype=np.float32))
    k_w = np.ascontiguousarray(np.asarray(k_w, dtype=np.float32))
    v_w = np.ascontiguousarray(np.asarray(v_w, dtype=np.float32))
    o_w = np.ascontiguousarray(np.asarray(o_w, dtype=np.float32))

    if "nc" not in _CACHE:
        _CACHE["nc"] = build_program()
    nc = _CACHE["nc"]

    in_maps = make_in_maps(hidden, q_w, k_w, v_w, o_w)
    from concourse.bass_utils import run_bass_kernel_spmd
    res = run_bass_kernel_spmd(nc, in_maps, core_ids=list(range(NC)))
    out = np.zeros((B, T, D), dtype=np.float32)
    for c in range(NC):
        out[c // 4] += res.results[c]["out"].astype(np.float32)
    return out


if __name__ == "__main__":
    print("building program...")
    nc = build_program()
    print("BUILD OK")


# revision 29
# speedup vs baseline: 2.0211x; 1.0060x over previous
"""Trainium2 Bass kernel for GroupedQueryAttention with 1-bit quantized linears.

Sharding (v2): 8 cores = 2 batches x 4 head-groups (tensor-parallel over
heads).  Core c handles batch b=c//4 and head-group hg=c%4: query heads
4hg..4hg+3, kv head hg, ALL 2048 tokens.  The output projection is computed
over the local 512 attention-output columns only -> each core emits a partial
[T, D] sum; the host adds the 4 partials per batch.  No K/V compute
replication (the v1 baseline recomputed full K/V on 4 cores each).

Per-core FLOPs drop 23.6 -> 17.2 GF and every matmul runs 512 output
columns (single PSUM bank) with contraction 128, emitted as one dense
back-to-back PE stream:

 - The PE HAM clock gate defaults to 1.2 GHz and only reaches 2.4 GHz after
   ~3.4us of continuous busy; idle windows re-throttle.  The v1 kernel ran
   mostly cold.  Here the PE stream is kept dense (software-pipelined
   attention, per-512-chunk projection evacuation, double-buffered PSUM) and
   dummy matmuls fill the DMA-bound first ~30us to hold the gate open.
 - Weights arrive host-pre-transposed (din-major); 1-bit quantization runs
   in that layout: scale = ones-matmul partition-reduce of |w| (broadcast to
   all partitions), then sign (ACT) * scale (DVE) in place.  No DRAM round
   trip, no on-device weight transposes.
 - Attention per (head, 512-query quarter): scores[k,q] via K-tile
   stationary, one strided exp over both heads' PSUM banks, triangular
   dmask on diagonal tiles, softmax denominator via ones-matmul (sum lands
   broadcast on all partitions -> wide fast reciprocal), V accumulation in
   PSUM, normalize on evac.
 - O-projection: OT stationary x quantized owT moving, 4x512-col chunks,
   bf16 partial out streamed to DRAM per 128-row tile.

Program is identical across cores; all per-core variation is input data.
"""

import sys

sys.path.insert(0, "/opt/trn_rl_repo")

import numpy as np
import ml_dtypes

import concourse.bacc as bacc
import concourse.bass as bass
import concourse.mybir as mybir
import concourse.tile as tile

F32 = mybir.dt.float32
F16 = mybir.dt.float16
BF16 = mybir.dt.bfloat16

B, T, D = 2, 2048, 2048
H, HK, HD = 16, 4, 128
G = 128
THETA = 1000000.0
NC = 8
HL = H // 4          # 4 local query heads per core
DT = D // 128        # 16 din tiles
NKT = T // 128       # 16 key tiles
NQC = T // 512       # 4 query quarters

ALPHA_Q = HD ** -0.5

Exp = mybir.ActivationFunctionType.Exp
Abs = mybir.ActivationFunctionType.Abs
MULT = mybir.AluOpType.mult
ADD = mybir.AluOpType.add
ABSMAX = mybir.AluOpType.abs_max


def _bcast(ap_small, like_ap):
    a, b = bass.broadcast_tensor_aps(like_ap, ap_small)
    return b


def build_program():
    nc = bacc.Bacc("TRN2", target_bir_lowering=False, debug=False, num_devices=NC)

    xT = nc.dram_tensor("xT", [D, T], BF16, kind="ExternalInput").ap()
    qwT = nc.dram_tensor("qwT", [D, HL * HD], BF16, kind="ExternalInput").ap()
    kwT = nc.dram_tensor("kwT", [D, HD], BF16, kind="ExternalInput").ap()
    vwT = nc.dram_tensor("vwT", [D, HD], BF16, kind="ExternalInput").ap()
    owT = nc.dram_tensor("owT", [HL * HD, D], BF16, kind="ExternalInput").ap()
    cosq = nc.dram_tensor("cosq", [HD, T], F16, kind="ExternalInput").ap()
    sinqr = nc.dram_tensor("sinqr", [HD, T], F16, kind="ExternalInput").ap()
    cosk = nc.dram_tensor("cosk", [HD, T], F16, kind="ExternalInput").ap()
    sinkr = nc.dram_tensor("sinkr", [HD, T], F16, kind="ExternalInput").ap()
    trimask = nc.dram_tensor("trimask", [128, 128], BF16, kind="ExternalInput").ap()
    out = nc.dram_tensor("out", [T, D], BF16, kind="ExternalOutput").ap()

    with tile.TileContext(nc) as tc:
        build_tile_kernel(nc, tc, xT, qwT, kwT, vwT, owT, cosq, sinqr, cosk,
                          sinkr, trimask, out)
    nc.compile()
    return nc


def build_tile_kernel(nc, tc, xT, qwT, kwT, vwT, owT, cosq, sinqr, cosk,
                      sinkr, trimask, out):
    from contextlib import ExitStack

    ctx = ExitStack()
    with ctx:
        const = ctx.enter_context(tc.tile_pool(name="const", bufs=1))
        resid = ctx.enter_context(tc.tile_pool(name="resid", bufs=1))
        rtmp = ctx.enter_context(tc.tile_pool(name="rtmp", bufs=1))

        # [128,128] of 1/G: ones-matmul over a probs tile gives the softmax
        # denominator broadcast on all 128 partitions; over |w| it gives the
        # group-mean quant scale broadcast likewise.  (1/G exact in bf16.)
        ones = const.tile([128, 128], BF16)
        nc.gpsimd.memset(ones, 1.0 / G)
        ones1 = const.tile([128, 128], BF16)
        nc.gpsimd.memset(ones1, 1.0)
        warm_src = const.tile([128, 512], BF16)
        nc.gpsimd.memset(warm_src, 0.0)
        tri_sb = const.tile([128, 128], BF16)

        cosq_sb = const.tile([128, T], F16)
        sinqr_sb = const.tile([128, T], F16)
        cosk_sb = const.tile([128, T], F16)
        sinkr_sb = const.tile([128, T], F16)

        # residents
        KTl = resid.tile([128, T], BF16)            # roped k^T  [kd, t]
        QTl = resid.tile([128, HL, T], BF16)        # roped q^T  [dh, h, t]
        Vl = resid.tile([128, NKT, HD], BF16)       # v row-major [t, kt, vd]
        VTs = resid.tile([128, T], BF16)            # v^T staging [vd, t]
        OT = resid.tile([128, HL, T], BF16)         # attn out^T [dh, h, q]

        # ---------------- input DMA (priority order) --------------------
        nc.sync.dma_start(tri_sb, trimask)
        with tc.tile_pool(name="wstage", bufs=1) as wst, \
             tc.tile_pool(name="xstage", bufs=1) as xst:
            kw_sb = wst.tile([128, DT, HD], BF16)
            vw_sb = wst.tile([128, DT, HD], BF16)
            qw_sb = wst.tile([128, DT, HL * HD], BF16)
            ow_sb = wst.tile([128, HL, D], BF16)
            XT = xst.tile([128, DT, T], BF16)

            nc.sync.dma_start(kw_sb, kwT.rearrange("(dt p) r -> p dt r", p=128))
            nc.sync.dma_start(vw_sb, vwT.rearrange("(dt p) r -> p dt r", p=128))
            nc.sync.dma_start(qw_sb, qwT.rearrange("(dt p) r -> p dt r", p=128))
            xsrc = xT.rearrange("(dt p) t -> p dt t", p=128)
            for dt in range(DT):
                nc.sync.dma_start(XT[:, dt, :], xsrc[:, dt, :])
            nc.sync.dma_start(cosk_sb, cosk)
            nc.sync.dma_start(sinkr_sb, sinkr)
            nc.sync.dma_start(cosq_sb, cosq)
            nc.sync.dma_start(sinqr_sb, sinqr)
            nc.sync.dma_start(ow_sb, owT.rearrange("(j p) d -> p j d", p=128))

            run_compute(nc, tc, ctx, const, resid, rtmp, ones, ones1,
                        warm_src, tri_sb, cosq_sb, sinqr_sb, cosk_sb,
                        sinkr_sb, kw_sb, vw_sb, qw_sb, ow_sb, XT, KTl, QTl,
                        Vl, VTs, OT, out)


def run_compute(nc, tc, ctx, const, resid, rtmp, ones, ones1, warm_src,
                tri_sb, cosq_sb, sinqr_sb, cosk_sb, sinkr_sb, kw_sb, vw_sb,
                qw_sb, ow_sb, XT, KTl, QTl, Vl, VTs, OT, out):

    def rope_copy(ps, w, eng="act"):
        """Evacuate PSUM -> SBUF f32 (frees the PSUM bank)."""
        pse = rtmp.tile([128, w], F32, tag="pse", bufs=4)
        if eng == "dve":
            nc.vector.tensor_copy(pse, ps)
        else:
            nc.scalar.copy(pse, ps)
        return pse

    def rope_finish(pse, cos_sb, sinr_sb, col0, w, out_ap):
        """out = pse*cos + rot(pse)*sinr (bf16).  DVE multiplies, gpsimd
        adds.  sinr tables arrive pre-rolled 64 partitions for
        base-partition legality."""
        t1 = rtmp.tile([128, w], F32, tag="t1", bufs=2)
        t2 = rtmp.tile([128, w], F32, tag="t2", bufs=2)
        cs = cos_sb[:, col0:col0 + w]
        sr = sinr_sb[:, col0:col0 + w]
        nc.vector.tensor_tensor(t1, pse, cs, op=MULT)
        nc.vector.tensor_tensor(t2[0:64, :], pse[64:128, :], sr[64:128, :],
                                op=MULT)
        nc.vector.tensor_tensor(t2[64:128, :], pse[0:64, :], sr[0:64, :],
                                op=MULT)
        nc.gpsimd.tensor_tensor(out_ap, t1, t2, op=ADD)

    def rope_evac(ps, cos_sb, sinr_sb, col0, w, out_ap, copy_eng="act"):
        pse = rope_copy(ps, w, copy_eng)
        rope_finish(pse, cos_sb, sinr_sb, col0, w, out_ap)

    # ============ phase A: quant small weights + K projection ============
    with tc.tile_pool(name="psA", bufs=1, space="PSUM") as psA, \
         tc.tile_pool(name="qtmp", bufs=1) as qtmp:

        def warm(n):
            """Dummy matmuls: keep the PE HAM clock gate open while the
            stream is DMA-paced.  Output never read."""
            for _ in range(n):
                wps = psA.tile([128, 512], F32, tag="warm", bufs=1)
                nc.tensor.matmul(wps, ones, warm_src, start=True, stop=True)

        def quant(w_sb, ncols, tag):
            """1-bit quantize a [128, ncols] din-major slab in place.
            Partition dim = one full quant group (G=128)."""
            ab = qtmp.tile([128, 512], BF16, tag="ab", bufs=2)
            for c0 in range(0, ncols, 512):
                w = min(512, ncols - c0)
                src = w_sb[:, c0:c0 + w]
                nc.scalar.activation(ab[:, 0:w], src, Abs)
                sc = psA.tile([128, 512], F32, tag="scale", bufs=2)
                nc.tensor.matmul(sc[:, 0:w], ones, ab[:, 0:w], start=True,
                                 stop=True)
                nc.scalar.sign(src, src)
                nc.vector.tensor_tensor(src, src, sc[:, 0:w], op=MULT)

        # k/v weight quant (chains overlap the weight DMAs)
        warm(8)
        quant(kw_sb.rearrange("p dt r -> p (dt r)"), DT * HD, "kw")
        quant(vw_sb.rearrange("p dt r -> p (dt r)"), DT * HD, "vw")
        warm(4)

        # K projection, dt-outer (paced by XT chunk arrival; dummy matmuls
        # fill the DMA slack so the clock gate stays open).  The qw quant
        # matmuls go AFTER the K loop: their ACT abs chains trail the DMA,
        # and a stalled quant matmul in the in-order PE queue would block
        # every K matmul behind it.
        qflat = qw_sb.rearrange("p dt r -> p (dt r)")
        psK = psA.tile([128, T], F32, tag="psK")
        for dt in range(DT):
            for cc in range(NQC):
                nc.tensor.matmul(psK[:, 512 * cc:512 * (cc + 1)],
                                 kw_sb[:, dt, :],
                                 XT[:, dt, 512 * cc:512 * (cc + 1)],
                                 start=(dt == 0), stop=(dt == DT - 1))
            warm(1)
        for dt in range(DT):
            quant(qflat[:, 512 * dt:512 * (dt + 1)], 512, "qw")
        # free all four psK banks first (copies split across ACT and DVE)
        # so phase B, whose PSUM tiles reuse these banks, starts promptly;
        # the rope math trails behind on DVE/gpsimd.
        pses = [rope_copy(psK[:, 512 * cc:512 * (cc + 1)], 512,
                          "dve" if cc % 2 else "act") for cc in range(NQC)]
        for cc in range(NQC):
            rope_finish(pses[cc], cosk_sb, sinkr_sb, 512 * cc, 512,
                        KTl[:, 512 * cc:512 * (cc + 1)])

    # ============ phase B: Q heads + V projection + ow quant =============
    with tc.tile_pool(name="psB", bufs=1, space="PSUM") as psB, \
         tc.tile_pool(name="qtmpB", bufs=1) as qtmpB:

        def quantB(src):
            ab = qtmpB.tile([128, 512], BF16, tag="ab", bufs=2)
            nc.scalar.activation(ab, src, Abs)
            sc = psB.tile([128, 512], F32, tag="scale", bufs=2)
            nc.tensor.matmul(sc, ones, ab, start=True, stop=True)
            nc.scalar.sign(src, src)
            nc.vector.tensor_tensor(src, src, sc, op=MULT)

        ow_chunks = [(j, c0) for j in range(HL) for c0 in range(0, D, 512)]
        owi = 0

        def ow_quant_step(n):
            nonlocal owi
            for _ in range(n):
                if owi >= len(ow_chunks):
                    return
                j, c0 = ow_chunks[owi]
                owi += 1
                quantB(ow_sb[:, j, c0:c0 + 512])

        # Q streams (cc-outer: 16 back-to-back 512-col matmuls per chunk)
        for h in range(HL):
            for cc in range(NQC):
                ps = psB.tile([128, 512], F32, tag="acc", bufs=3)
                for dt in range(DT):
                    nc.tensor.matmul(ps, qw_sb[:, dt, 128 * h:128 * (h + 1)],
                                     XT[:, dt, 512 * cc:512 * (cc + 1)],
                                     start=(dt == 0), stop=(dt == DT - 1))
                rope_evac(ps, cosq_sb, sinqr_sb, 512 * cc, 512,
                          QTl[:, h, 512 * cc:512 * (cc + 1)])
                if h >= 1:
                    ow_quant_step(2)

        # V projection -> V^T, then XBAR-transpose to row-major V tiles
        for cc in range(NQC):
            ps = psB.tile([128, 512], F32, tag="acc", bufs=3)
            for dt in range(DT):
                nc.tensor.matmul(ps, vw_sb[:, dt, :],
                                 XT[:, dt, 512 * cc:512 * (cc + 1)],
                                 start=(dt == 0), stop=(dt == DT - 1))
            nc.scalar.copy(VTs[:, 512 * cc:512 * (cc + 1)], ps)
            nc.sync.dma_start_transpose(Vl[:, 4 * cc:4 * (cc + 1), :],
                                        VTs[:, 512 * cc:512 * (cc + 1)])
            ow_quant_step(1)
        ow_quant_step(99)

    # ===================== phase C: attention ===========================
    with tc.tile_pool(name="psC", bufs=1, space="PSUM") as psC, \
         tc.tile_pool(name="apool", bufs=1) as apool:
        # software-pipelined across (head-pair, query-quarter) units: the
        # accumulation matmuls of step k are deferred until step k+1's
        # scores are in flight, so the PE never waits on exp directly.
        pend = []

        def drain_one():
            while pend:
                kind, fn = pend.pop(0)
                fn()
                if kind == "acc":
                    break

        def mk_acc(po, pd, kt, nkt, qoff, pt):
            def go():
                first, last = kt == 0, kt == nkt - 1
                for hh in range(2):
                    nc.tensor.matmul(pd[:, hh, qoff:], ones1,
                                     pt[:, hh, qoff:], start=first, stop=last)
                    nc.tensor.matmul(po[:, hh, qoff:], Vl[:, kt, :],
                                     pt[:, hh, qoff:], start=first, stop=last)
            return go

        def mk_fin(po, pd, h0, q0):
            def go():
                rq = apool.tile([128, 2, 512], F32, tag="rq", bufs=2)
                nc.vector.reciprocal_approx_fast(rq, pd)
                nc.vector.tensor_tensor(OT[:, h0:h0 + 2, q0:q0 + 512], po, rq,
                                        op=MULT)
            return go

        for hp in range(HL // 2):
            h0 = 2 * hp
            for m in range(NQC):
                q0 = 512 * m
                nkt = 4 * (m + 1)
                po = psC.tile([128, 2, 512], F32, tag="po")
                pd = psC.tile([128, 2, 512], F32, tag="pd")

                for kt in range(nkt):
                    kc = 128 * kt
                    dj = kt - 4 * m
                    qoff = 128 * dj if dj >= 0 else 0
                    st = psC.tile([128, 2, 512], F32, tag="st", bufs=2)
                    for hh in range(2):
                        nc.tensor.matmul(st[:, hh, qoff:],
                                         KTl[:, kc:kc + 128],
                                         QTl[:, h0 + hh, q0 + qoff:q0 + 512],
                                         start=True, stop=True)
                    pt = apool.tile([128, 2, 512], BF16, tag="pt", bufs=3)
                    nc.scalar.activation(pt[:, :, qoff:], st[:, :, qoff:],
                                         Exp)
                    if dj >= 0:
                        blk = slice(qoff, qoff + 128)
                        for hh in range(2):
                            nc.vector.tensor_tensor(pt[:, hh, blk],
                                                    pt[:, hh, blk], tri_sb,
                                                    op=MULT)
                    drain_one()
                    pend.append(("acc", mk_acc(po, pd, kt, nkt, qoff, pt)))
                pend.append(("fin", mk_fin(po, pd, h0, q0)))
        while pend:
            pend.pop(0)[1]()

    # ==================== phase D: output projection ====================
    # half-row-tiles (2 banks, bufs=2 = 4 banks): starts as soon as half
    # of PSUM frees, and the 8MB output streams out in 256KB pieces on two
    # alternating DGE queues (SP / ACT) so no ring backlogs into a tail.
    with tc.tile_pool(name="psD", bufs=1, space="PSUM") as psD, \
         tc.tile_pool(name="opool", bufs=1) as opool:
        for qt in range(NKT):
            for half in range(2):
                op = psD.tile([128, 1024], F32, tag="op", bufs=2)
                for cc in range(2):
                    c0 = 1024 * half + 512 * cc
                    for ht in range(HL):
                        nc.tensor.matmul(op[:, 512 * cc:512 * (cc + 1)],
                                         OT[:, ht, 128 * qt:128 * (qt + 1)],
                                         ow_sb[:, ht, c0:c0 + 512],
                                         start=(ht == 0), stop=(ht == HL - 1))
                osb = opool.tile([128, 1024], BF16, tag="osb", bufs=4)
                nc.vector.tensor_copy(osb[:, 0:512], op[:, 0:512])
                nc.scalar.copy(osb[:, 512:1024], op[:, 512:1024])
                eng = nc.sync if half == 0 else nc.scalar
                eng.dma_start(out[128 * qt:128 * (qt + 1),
                                  1024 * half:1024 * (half + 1)], osb)


# ---------------------------------------------------------------------------
# host side
# ---------------------------------------------------------------------------
_CACHE = {}


def _tables():
    inv = 1.0 / (THETA ** (np.arange(0, HD, 2, dtype=np.float64) / HD))
    t = np.arange(T, dtype=np.float64)
    fr = np.outer(t, inv)                      # [T, 64]
    emb = np.concatenate([fr, fr], axis=1)     # [T, 128]
    cosT = np.cos(emb).T                       # [128, T] float64
    sinT = np.sin(emb).T
    sinr = np.empty_like(sinT)
    sinr[0:64] = -sinT[0:64]
    sinr[64:128] = sinT[64:128]
    # rolled by 64 partitions: kernel reads sr[64:128] for out[0:64] etc.
    sinr = np.roll(sinr, 64, axis=0)
    return cosT, sinr


def make_in_maps(hidden, q_w, k_w, v_w, o_w):
    cosT, sinr = _tables()
    f16 = np.float16
    bf = ml_dtypes.bfloat16
    cq = np.ascontiguousarray(cosT * ALPHA_Q).astype(f16)
    sq = np.ascontiguousarray(sinr * ALPHA_Q).astype(f16)
    ck = np.ascontiguousarray(cosT).astype(f16)
    sk = np.ascontiguousarray(sinr).astype(f16)
    tri = (np.arange(128)[:, None] <= np.arange(128)[None, :]).astype(bf)
    in_maps = []
    for c in range(NC):
        b, hg = c // 4, c % 4
        in_maps.append({
            "xT": np.ascontiguousarray(hidden[b].T.astype(bf)),
            "qwT": np.ascontiguousarray(
                q_w[512 * hg:512 * (hg + 1), :].T.astype(bf)),
            "kwT": np.ascontiguousarray(
                k_w[128 * hg:128 * (hg + 1), :].T.astype(bf)),
            "vwT": np.ascontiguousarray(
                v_w[128 * hg:128 * (hg + 1), :].T.astype(bf)),
            "owT": np.ascontiguousarray(
                o_w[:, 512 * hg:512 * (hg + 1)].T.astype(bf)),
            "cosq": cq, "sinqr": sq, "cosk": ck, "sinkr": sk,
            "trimask": tri,
        })
    return in_maps


def kernel(hidden, q_w, k_w, v_w, o_w):
    hidden = np.asarray(hidden, dtype=np.float32)
    q_w = np.ascontiguousarray(np.asarray(q_w, dtype=np.float32))
    k_w = np.ascontiguousarray(np.asarray(k_w, dtype=np.float32))
    v_w = np.ascontiguousarray(np.asarray(v_w, dtype=np.float32))
    o_w = np.ascontiguousarray(np.asarray(o_w, dtype=np.float32))

    if "nc" not in _CACHE:
        _CACHE["nc"] = build_program()
    nc = _CACHE["nc"]

    in_maps = make_in_maps(hidden, q_w, k_w, v_w, o_w)
    from concourse.bass_utils import run_bass_kernel_spmd
    res = run_bass_kernel_spmd(nc, in_maps, core_ids=list(range(NC)))
    out = np.zeros((B, T, D), dtype=np.float32)
    for c in range(NC):
        out[c // 4] += res.results[c]["out"].astype(np.float32)
    return out


if __name__ == "__main__":
    print("building program...")
    nc = build_program()
    print("BUILD OK")
